# revision 1
# baseline (speedup 1.0000x reference)
"""Trainium2 Bass kernel for nn_Attention_9096740733536 (sparse_attention).

Sharding: data-parallel over the QB (task) dim across 8 cores (2 tasks/core),
one mid-kernel AllReduce of [feat_corr partials | q_global | k_global] sums.
The attention math is algebraically collapsed: mixed scores are linear (no
softmax), so
  out[h,q] = alpha_h*(Fq/qn) @ ((Fk/kn)^T @ Fv) + ww_h*q_ratio (x) (k_ratio^T Fv)
with 128x128 inner matrices instead of 512x512 score matrices, and layernorm
is folded into the input projection via rank-1 PSUM augmentation.
"""
import numpy as np
from contextlib import ExitStack

import concourse.bass as bass
import concourse.tile as tile
from concourse import bacc, mybir
from concourse import bass_utils
from concourse._compat import with_exitstack

F32 = mybir.dt.float32
F32R = mybir.dt.float32r
AF = mybir.ActivationFunctionType
ALU = mybir.AluOpType
AX = mybir.AxisListType

H, D, DIM = 8, 128, 1024
QB, N = 16, 512
N_CORES = 8
T = QB * N // N_CORES          # 1024 tokens per core
NT = T // 128                  # 8 token tiles per core
NTASK = T // N                 # 2 tasks per core
LN_EPS = 1e-5
TOK_ALL = float(QB * N)


@with_exitstack
def attn_kernel(ctx: ExitStack, tc: tile.TileContext, outs, ins, n_cores=N_CORES):
    nc = tc.nc
    y = outs[0]
    (xn_q, xn_k, xn_v, xT_q, xT_k, xT_v, Wp_d, WoT_d, negu_d, vrow_d,
     bout_d, ones_d, ident_d, mask_d, wp1T_d, wp2T_d, b1_d, gbc_d, bbc_d,
     b2bc_d) = ins

    consts = ctx.enter_context(tc.tile_pool(name="consts", bufs=1))
    fpool = ctx.enter_context(tc.tile_pool(name="fpool", bufs=1))
    stat1 = ctx.enter_context(tc.tile_pool(name="stat1", bufs=1))
    dram = ctx.enter_context(tc.tile_pool(name="dram", bufs=1, space="DRAM"))

    ps_proj = ctx.enter_context(tc.tile_pool(name="ps_proj", bufs=3, space="PSUM"))
    ps_fc = ctx.enter_context(tc.tile_pool(name="ps_fc", bufs=2, space="PSUM"))
    ps_gk = ctx.enter_context(tc.tile_pool(name="ps_gk", bufs=1, space="PSUM"))
    ps_o1 = ctx.enter_context(tc.tile_pool(name="ps_o1", bufs=1, space="PSUM"))
    ps_small = ctx.enter_context(tc.tile_pool(name="ps_small", bufs=1, space="PSUM"))

    # ---- small constants (long-lived) ----
    ident = consts.tile([128, 128], F32)
    nc.sync.dma_start(ident[:], ident_d[:])
    bout = consts.tile([1, DIM], F32R)
    nc.sync.dma_start(bout[:], bout_d[:].bitcast(F32R))
    onesr = consts.tile([1, 128], F32R)
    nc.sync.dma_start(onesr[:], ones_d[0:1, :].bitcast(F32R))
    ones = consts.tile([128, 8], F32)
    nc.sync.dma_start(ones[:], ones_d[:, 0:8])
    mask_nd = consts.tile([128, H * 128], F32)
    nc.scalar.dma_start(mask_nd[:], mask_d[:])
    wp1T = consts.tile([128, 256], F32)
    nc.scalar.dma_start(wp1T[:], wp1T_d[:])
    wp2T = consts.tile([128, 3], F32)
    nc.scalar.dma_start(wp2T[:], wp2T_d[:])
    b1row = consts.tile([1, 128], F32)
    nc.scalar.dma_start(b1row[:], b1_d[:])
    ones8 = consts.tile([1, 8], F32)
    nc.sync.dma_start(ones8[:], ones_d[0:1, 0:8])
    gbc = consts.tile([8, 128], F32)
    nc.scalar.dma_start(gbc[:], gbc_d[:])
    bbc = consts.tile([8, 128], F32)
    nc.scalar.dma_start(bbc[:], bbc_d[:])
    b2bc = consts.tile([8, 3], F32)
    nc.scalar.dma_start(b2bc[:], b2bc_d[:])
    eps = consts.tile([128, 1], F32)
    nc.vector.memset(eps[:], LN_EPS)

    # ---- persistent F tensors: [128 tok, t*1024 + h*128 + d] ----
    Fq = fpool.tile([128, NT * DIM], F32)
    Fk = fpool.tile([128, NT * DIM], F32)
    Fv = fpool.tile([128, NT * DIM], F32)
    sq_scr = stat1.tile([128, DIM], F32)     # ACT square scratch (write-only)

    xns = [xn_q, xn_k, xn_v]
    xTs = [xT_q, xT_k, xT_v]
    Fs = [Fq, Fk, Fv]

    # ======== Phase 1: folded-LN projection (scoped pools) ========
    with tc.tile_pool(name="ph1", bufs=1) as ph1, \
         tc.tile_pool(name="xpool", bufs=3) as xpool, \
         tc.tile_pool(name="spool", bufs=3) as spool:
        Wp = ph1.tile([128, 8 * DIM], F32R)
        for s in range(8):
            nc.gpsimd.dma_start(Wp[:, s * DIM:(s + 1) * DIM],
                                Wp_d[:, s * DIM:(s + 1) * DIM].bitcast(F32R))
        negu = ph1.tile([1, DIM], F32R)
        nc.sync.dma_start(negu[:], negu_d[:].bitcast(F32R))
        vrow = ph1.tile([1, DIM], F32R)
        nc.sync.dma_start(vrow[:], vrow_d[:].bitcast(F32R))
        for t in range(NT):
            st = spool.tile([128, 12], F32, tag="st")
            bn6 = spool.tile([128, 36], F32, tag="bn6")
            rsig = spool.tile([128, 3], F32, tag="rsig")
            for i in range(3):
                xn = xpool.tile([128, DIM], F32, tag="xn")
                nc.sync.dma_start(xn[:], xns[i][t * 128:(t + 1) * 128, :])
                nc.vector.bn_stats(bn6[:, i * 12:i * 12 + 6], xn[:, 0:512])
                nc.vector.bn_stats(bn6[:, i * 12 + 6:i * 12 + 12],
                                   xn[:, 512:1024])
                # (mean, var) pair -> st cols (6+i, 9+i via sqrt)
                nc.vector.bn_aggr(st[:, 2 * i:2 * i + 2],
                                  bn6[:, i * 12:i * 12 + 12])
            # st cols 0,2,4 = mu ; 1,3,5 = var
            nc.vector.tensor_copy(st[:, 6:9], st[:, 0:6:2])
            nc.scalar.activation(st[:, 9:12], st[:, 1:6:2], AF.Sqrt,
                                 bias=eps[:])
            nc.vector.reciprocal(rsig[:], st[:, 9:12])
            # transpose [mu|sig] (cols 6..11) -> rows [6, 128] -> flat [1, 768]
            trp = ps_small.tile([6, 128], F32, tag="sm")
            nc.tensor.transpose(trp[:], st[:, 6:12], ident[:])
            rows6 = spool.tile([6, 128], F32R, tag="rows6")
            nc.scalar.copy(rows6[:], trp[:])
            rows = spool.tile([1, 768], F32R, tag="rows")
            nc.scalar.dma_start(rows[:], rows6[:])
            for i in range(3):
                xT_t = xpool.tile([128, DIM], F32R, tag="xT")
                nc.sync.dma_start(xT_t[:],
                                  xTs[i][:, t * DIM:(t + 1) * DIM].bitcast(F32R))
                for half in range(2):
                    o = half * 512
                    acc = ps_proj.tile([128, 512], F32, tag="proj")
                    for s in range(8):
                        nc.tensor.matmul(
                            acc[:], xT_t[:, s * 128:(s + 1) * 128],
                            Wp[:, s * DIM + o: s * DIM + o + 512],
                            start=(s == 0), stop=False)
                    nc.tensor.matmul(acc[:], rows[:, i * 128:(i + 1) * 128],
                                     negu[:, o:o + 512], start=False, stop=False)
                    nc.tensor.matmul(acc[:], rows[:, (3 + i) * 128:(4 + i) * 128],
                                     vrow[:, o:o + 512], start=False, stop=True)
                    dst = Fs[i][:, t * DIM + o: t * DIM + o + 512]
                    if (i + half) % 2 == 0:
                        nc.scalar.mul(dst, acc[:], rsig[:, i:i + 1])
                    else:
                        nc.vector.tensor_scalar_mul(dst, acc[:],
                                                    rsig[:, i:i + 1])

    # ======== Phase 2: F stats, feat_corr partials, q/k globals ========
    late = ctx.enter_context(tc.tile_pool(name="late", bufs=1))
    WoT = late.tile([128, 8 * DIM], F32R)
    nc.gpsimd.dma_start(WoT[:], WoT_d[:].bitcast(F32R))

    qss = stat1.tile([128, 64], F32)   # col t*8+h : sumsq over d of Fq
    qsm = stat1.tile([128, 64], F32)   # sums over d
    kss = stat1.tile([128, 64], F32)
    ksm = stat1.tile([128, 64], F32)
    qmean = stat1.tile([128, 64], F32)
    qninv = stat1.tile([128, 64], F32)
    kninv = stat1.tile([128, 64], F32)
    kn = stat1.tile([128, 64], F32)
    qr = stat1.tile([128, 64], F32)
    kr = stat1.tile([128, 64], F32)
    rscr = stat1.tile([128, 96], F32)  # ratio-chain scratch (3x32 per half)

    def derived(ss, sm, ninv, ratio, s, n_out=None):
        # ninv = 1/sqrt(ss); var = ss/127 - sm^2/(128*127)
        # ratio = 2*min(var,1)/(var+1)
        w = s.stop - s.start
        if n_out is not None:
            nc.scalar.activation(n_out[:, s], ss[:, s], AF.Sqrt)
            nc.vector.reciprocal(ninv[:, s], n_out[:, s])
        else:
            nc.scalar.activation(ninv[:, s], ss[:, s], AF.Sqrt)
            nc.vector.reciprocal(ninv[:, s], ninv[:, s])
        t1 = rscr[:, 0:w]
        nc.vector.tensor_tensor(t1, sm[:, s], sm[:, s], op=ALU.mult)
        nc.vector.tensor_scalar_mul(t1, t1, 1.0 / (D * (D - 1)))
        t2 = rscr[:, w:2 * w]
        nc.vector.tensor_scalar_mul(t2, ss[:, s], 1.0 / (D - 1))
        var = rscr[:, 2 * w:3 * w]
        nc.vector.tensor_tensor(var, t2, t1, op=ALU.subtract)
        nc.vector.tensor_scalar(t1, var, 1.0, 2.0, ALU.min, ALU.mult)
        nc.vector.tensor_scalar_add(t2, var, 1.0)
        nc.vector.reciprocal(t2, t2)
        nc.vector.tensor_tensor(ratio[:, s], t1, t2, op=ALU.mult)

    for jh in range(NTASK):
        for t in range(4 * jh, 4 * jh + 4):
            nc.vector.reduce_sum(
                qsm[:, t * 8:(t + 1) * 8],
                Fq[:, t * DIM:(t + 1) * DIM].rearrange("p (h d) -> p h d", h=8),
                axis=AX.X)
            nc.vector.reduce_sum(
                ksm[:, t * 8:(t + 1) * 8],
                Fk[:, t * DIM:(t + 1) * DIM].rearrange("p (h d) -> p h d", h=8),
                axis=AX.X)
            for h in range(H):
                sl = slice(t * DIM + h * 128, t * DIM + h * 128 + 128)
                nc.scalar.activation(sq_scr[:, 0:128], Fq[:, sl], AF.Square,
                                     accum_out=qss[:, t * 8 + h:t * 8 + h + 1])
                nc.scalar.activation(sq_scr[:, 128:256], Fk[:, sl], AF.Square,
                                     accum_out=kss[:, t * 8 + h:t * 8 + h + 1])
        s = slice(jh * 32, jh * 32 + 32)
        # NOTE: qmean holds NEGATED means (used as ACT bias for centering)
        nc.vector.tensor_scalar_mul(qmean[:, s], qsm[:, s], -1.0 / D)
        derived(qss, qsm, qninv, qr, s)
        derived(kss, ksm, kninv, kr, s, n_out=kn)
        # absorb kn into k_ratio: mv uses scaled Fv, so kr must carry kn back
        nc.vector.tensor_tensor(kr[:, s], kr[:, s], kn[:, s], op=ALU.mult)
        # scale Fv in place by 1/kn (only consumer is the M/mv stage)
        for t in range(4 * jh, 4 * jh + 4):
            for h in range(H):
                sl = slice(t * DIM + h * 128, t * DIM + h * 128 + 128)
                nc.vector.tensor_scalar(Fv[:, sl], Fv[:, sl],
                                        kninv[:, t * 8 + h:t * 8 + h + 1],
                                        None, ALU.mult)

    # ======== Phase 4a: allreduce-independent M/mv stage ========
    # M = Fk^T @ (Fv/kn) and mv = (kr*kn)^T @ (Fv/kn) per (head, task),
    # evicted UNSCALED (alpha/ww applied post-allreduce). Placed BEFORE the
    # feat_corr stage so the in-order PE stream overlaps the phase-1 tail.
    attn = ctx.enter_context(tc.tile_pool(name="attn", bufs=1))
    mm_raw = {}
    mv_raw = {}
    for j in range(NTASK):
        for h in range(H):
            mm_ps = ps_fc.tile([128, 128], F32, tag="fc128", name="mm_ps")
            mv_ps = ps_small.tile([1, 128], F32, tag="sm", name="mv_ps")
            for ti in range(4):
                t = 4 * j + ti
                sl = slice(t * DIM + h * 128, t * DIM + h * 128 + 128)
                nc.tensor.matmul(mm_ps[:], Fk[:, sl], Fv[:, sl],
                                 start=(ti == 0), stop=(ti == 3))
                nc.tensor.matmul(mv_ps[:], kr[:, t * 8 + h:t * 8 + h + 1],
                                 Fv[:, sl], start=(ti == 0), stop=(ti == 3))
            mm = attn.tile([128, 128], F32R, tag=f"mm{h}{j}", name="mm")
            nc.scalar.copy(mm[:], mm_ps[:])
            mv = attn.tile([1, 128], F32R, tag=f"mv{h}{j}", name="mv")
            nc.scalar.copy(mv[:], mv_ps[:])
            mm_raw[(h, j)] = mm
            mv_raw[(h, j)] = mv

    # feat_corr partials (per head) + q/k global sums (single PSUM group)
    # t-outer emission so no engine stream blocks on the last proj tile.
    ar_in = dram.tile([128, H * 128 + 16], F32)
    ar_out = dram.tile([128, H * 128 + 16], F32)
    gk_ps = ps_gk.tile([128, 16], F32, tag="gk")
    with tc.tile_pool(name="ph2", bufs=2) as ph2, \
         tc.tile_pool(name="qcpool", bufs=64) as qcpool:
        qc_tiles = {}
        for t in range(NT):
            for h in range(H):
                sl = slice(t * DIM + h * 128, t * DIM + h * 128 + 128)
                qc = qcpool.tile([128, 128], mybir.dt.bfloat16, tag="qc",
                                 name="qc")
                nc.scalar.activation(qc[:], Fq[:, sl], AF.Identity,
                                     bias=qmean[:, t * 8 + h:t * 8 + h + 1])
                qc_tiles[(t, h)] = qc
                first = (h == 0 and t == 0)
                last = (h == H - 1 and t == NT - 1)
                nc.tensor.matmul(gk_ps[:, h:h + 1], Fq[:, sl], ones[:, 0:1],
                                 start=first, stop=last, skip_group_check=True)
                nc.tensor.matmul(gk_ps[:, 8 + h:9 + h], Fk[:, sl], ones[:, 0:1],
                                 start=False, stop=False, skip_group_check=True)
        for h in range(H):
            fc_ps = ps_fc.tile([128, 128], F32, tag="fc128", name="fc_ps")
            for t in range(NT):
                nc.tensor.matmul(fc_ps[:], qc_tiles[(t, h)][:],
                                 qc_tiles[(t, h)][:],
                                 start=(t == 0), stop=(t == NT - 1))
            fc_sb = ph2.tile([128, 128], F32, tag="fcsb", name="fc_sb")
            nc.vector.tensor_copy(fc_sb[:], fc_ps[:])
            nc.sync.dma_start(ar_in[:, h * 128:(h + 1) * 128], fc_sb[:])
        gk_sb = ph2.tile([128, 16], F32, tag="gksb", name="gk_sb")
        nc.scalar.copy(gk_sb[:], gk_ps[:])
        nc.sync.dma_start(ar_in[:, H * 128:H * 128 + 16], gk_sb[:])

    # in-place Fq <- Fq/qn (after feat_corr reads; gates only phase 4b)
    for h in range(H):
        for t in range(NT):
            sl = slice(t * DIM + h * 128, t * DIM + h * 128 + 128)
            c = slice(t * 8 + h, t * 8 + h + 1)
            nc.vector.tensor_scalar(Fq[:, sl], Fq[:, sl], qninv[:, c], None,
                                    ALU.mult)

    # ======== AllReduce ========
    if n_cores > 1:
        nc.gpsimd.collective_compute(
            "AllReduce", ALU.add,
            replica_groups=[list(range(n_cores))],
            ins=[ar_in.opt()], outs=[ar_out.opt()])
    else:  # single-core sim variant: allreduce over one core == copy
        nc.sync.dma_start(ar_out[:], ar_in[:])
    ar = late.tile([128, H * 128 + 16], F32)
    nc.sync.dma_start(ar[:], ar_out[:])
    arg = ar[:, H * 128:H * 128 + 16]

    # ======== Phase 3: decorr scale + weight predictor ========
    ssq = stat1.tile([128, 8], F32)
    msk = late.tile([128, H * 128], F32)
    nc.vector.tensor_tensor(msk[:], ar[:, 0:H * 128], mask_nd[:], op=ALU.mult)
    nc.scalar.activation(sq_scr[:, 0:H * 128], msk[:], AF.Square,
                         scale=1.0 / TOK_ALL)
    nc.vector.reduce_sum(ssq[:],
                         sq_scr[:, 0:H * 128].rearrange("p (h d) -> p h d", h=8),
                         axis=AX.X)
    ss_ps = ps_small.tile([8, 8], F32, tag="sm", name="ss_ps")
    nc.tensor.matmul(ss_ps[:], ssq[:], ones[:, 0:8], start=True, stop=True)
    dsc = stat1.tile([8, 8], F32)
    nc.scalar.activation(dsc[:, 0:1], ss_ps[0:8, 0:1], AF.Sqrt)
    nc.scalar.activation(dsc[:, 1:2], dsc[:, 0:1], AF.Exp, scale=-5.0 / (D * D))

    featsq = stat1.tile([128, 8], F32)
    nc.vector.tensor_scalar_mul(featsq[:], arg[:, 0:8], 1.0 / TOK_ALL)
    featsk = stat1.tile([128, 8], F32)
    nc.vector.tensor_scalar_mul(featsk[:], arg[:, 8:16], 1.0 / TOK_ALL)
    h1_ps = ps_small.tile([8, 128], F32, tag="sm", name="h1_ps")
    nc.tensor.matmul(h1_ps[:], featsq[:], wp1T[:, 0:128], start=True, stop=False)
    nc.tensor.matmul(h1_ps[:], featsk[:], wp1T[:, 128:256], start=False,
                     stop=False)
    nc.tensor.matmul(h1_ps[:], ones8[:], b1row[:], start=False, stop=True)
    h1 = stat1.tile([8, 128], F32)
    nc.scalar.copy(h1[:], h1_ps[:])
    w_mu = stat1.tile([8, 4], F32)
    nc.vector.reduce_sum(w_mu[:, 0:1], h1[:], axis=AX.X)
    nc.vector.tensor_scalar_mul(w_mu[:, 0:1], w_mu[:, 0:1], 1.0 / D)
    nc.scalar.activation(sq_scr[0:8, 0:128], h1[:], AF.Square,
                         accum_out=w_mu[:, 1:2])
    nc.vector.tensor_scalar_mul(w_mu[:, 1:2], w_mu[:, 1:2], 1.0 / D)
    nc.vector.tensor_tensor(w_mu[:, 2:3], w_mu[:, 0:1], w_mu[:, 0:1], op=ALU.mult)
    nc.vector.tensor_tensor(w_mu[:, 2:3], w_mu[:, 1:2], w_mu[:, 2:3],
                            op=ALU.subtract)
    nc.scalar.activation(w_mu[:, 3:4], w_mu[:, 2:3], AF.Sqrt, bias=eps[0:8, :])
    nc.vector.reciprocal(w_mu[:, 3:4], w_mu[:, 3:4])
    h1n = stat1.tile([8, 128], F32)
    nc.vector.tensor_scalar(h1n[:], h1[:], w_mu[:, 0:1], w_mu[:, 3:4],
                            ALU.subtract, ALU.mult)
    nc.vector.tensor_tensor(h1n[:], h1n[:], gbc[:], op=ALU.mult)
    nc.vector.tensor_tensor(h1n[:], h1n[:], bbc[:], op=ALU.add)
    nc.vector.tensor_scalar_max(h1n[:], h1n[:], 0.0)
    h1T_ps = ps_small.tile([128, 8], F32, tag="sm", name="h1T_ps")
    nc.tensor.transpose(h1T_ps[:], h1n[:], ident[0:8, 0:8])
    h1T = stat1.tile([128, 8], F32)
    nc.scalar.copy(h1T[:], h1T_ps[:])
    lg_ps = ps_small.tile([8, 3], F32, tag="sm", name="lg_ps")
    nc.tensor.matmul(lg_ps[:], h1T[:], wp2T[:], start=True, stop=True)
    lg = stat1.tile([8, 8], F32)
    nc.scalar.copy(lg[:, 0:3], lg_ps[:])
    nc.vector.tensor_tensor(lg[:, 0:3], lg[:, 0:3], b2bc[:], op=ALU.add)
    # logits are O(1): skip the (mathematically redundant) max-subtraction
    nc.scalar.activation(lg[:, 0:3], lg[:, 0:3], AF.Exp)
    nc.vector.reduce_sum(lg[:, 4:5], lg[:, 0:3], axis=AX.X)
    nc.vector.reciprocal(lg[:, 4:5], lg[:, 4:5])
    nc.vector.tensor_scalar(lg[:, 0:3], lg[:, 0:3], lg[:, 4:5], None, ALU.mult)
    # alpha = w0 + w1*dsc ; ww = w2 ; broadcast to 128 partitions
    aw = stat1.tile([8, 2], F32)
    nc.vector.tensor_tensor(aw[:, 0:1], lg[:, 1:2], dsc[:, 1:2], op=ALU.mult)
    nc.vector.tensor_tensor(aw[:, 0:1], aw[:, 0:1], lg[:, 0:1], op=ALU.add)
    nc.vector.tensor_copy(aw[:, 1:2], lg[:, 2:3])
    awT_ps = ps_small.tile([2, 8], F32, tag="sm", name="awT_ps")
    nc.tensor.transpose(awT_ps[:], aw[:], ident[0:8, 0:8])
    awT = stat1.tile([2, 8], F32)
    nc.scalar.copy(awT[:], awT_ps[:])
    aw_flat = stat1.tile([1, 16], F32)
    nc.scalar.dma_start(aw_flat[:], awT[:])
    abc = stat1.tile([128, 8], F32)
    nc.gpsimd.partition_broadcast(abc[:], aw_flat[:, 0:8])
    wbc = stat1.tile([128, 8], F32)
    nc.gpsimd.partition_broadcast(wbc[:], aw_flat[:, 8:16])

    # ======== Phase 4b + 5: scaled attention + output projection ========
    with tc.tile_pool(name="ph4", bufs=2) as ph4, \
         tc.tile_pool(name="o1pool", bufs=10) as o1pool:
        o1_tiles = {}
        for j in range(NTASK):
            for h in range(H):
                mm_sb = ph4.tile([128, 128], F32R, tag="mmsb", name="mm_sb")
                nc.vector.tensor_scalar(mm_sb[:], mm_raw[(h, j)][:],
                                        abc[:, h:h + 1], None, ALU.mult)
                mv_sb = ph4.tile([1, 128], F32R, tag="mvsb", name="mv_sb")
                nc.vector.tensor_scalar(mv_sb[:], mv_raw[(h, j)][:],
                                        wbc[0:1, h:h + 1], None, ALU.mult)

                # q_ratio row for this (h, j): [1, 512]
                c0 = 4 * j * 8 + h
                wq_ps = ps_small.tile([4, 128], F32, tag="sm", name="wq_ps")
                nc.tensor.transpose(wq_ps[:], qr[:, c0:c0 + 25:8], ident[:])
                wq4 = ph4.tile([4, 128], F32R, tag="wq4", name="wq4")
                nc.scalar.copy(wq4[:], wq_ps[:])
                wqr = ph4.tile([1, 512], F32R, tag="wqr", name="wqr")
                nc.scalar.dma_start(wqr[:], wq4[:])

                fqTs = ph4.tile([128, 512], F32R, tag="fqTs", name="fqTs")
                for ti in range(4):
                    t = 4 * j + ti
                    sl = slice(t * DIM + h * 128, t * DIM + h * 128 + 128)
                    qsT_ps = ps_fc.tile([128, 128], F32, tag="fc128",
                                        name="qsT_ps")
                    nc.tensor.transpose(qsT_ps[:], Fq[:, sl], ident[:])
                    nc.scalar.copy(fqTs[:, ti * 128:(ti + 1) * 128], qsT_ps[:])

                o1_ps = ps_o1.tile([128, 512], F32, tag="o1", name="o1_ps")
                nc.tensor.matmul(o1_ps[:], mm_sb[:], fqTs[:], start=True,
                                 stop=False)
                nc.tensor.matmul(o1_ps[:], mv_sb[:], wqr[:],
                                 start=False, stop=True)
                o1 = o1pool.tile([128, 512], F32R, tag="o1sb", name="o1_sb")
                nc.vector.tensor_copy(o1[:], o1_ps[:])
                o1_tiles[(h, j)] = o1

            # ---- output projection for this task ----
            for t in range(4 * j, 4 * j + 4):
                ti = t % 4
                for half in range(2):
                    o = half * 512
                    op_ps = ps_proj.tile([128, 512], F32, tag="proj",
                                         name="op_ps")
                    for h in range(H):
                        nc.tensor.matmul(
                            op_ps[:],
                            o1_tiles[(h, j)][:, ti * 128:(ti + 1) * 128],
                            WoT[:, h * DIM + o: h * DIM + o + 512],
                            start=(h == 0), stop=False)
                    nc.tensor.matmul(op_ps[:], onesr[:, 0:128],
                                     bout[:, o:o + 512],
                                     start=False, stop=True)
                    ysb = ph4.tile([128, 512], F32, tag="ysb", name="ysb")
                    nc.vector.tensor_copy(ysb[:], op_ps[:])
                    nc.sync.dma_start(y[t * 128:(t + 1) * 128, o:o + 512],
                                      ysb[:])


_BUILT = {}


def _build(n_cores=N_CORES):
    if n_cores in _BUILT:
        return _BUILT[n_cores]
    nc = bacc.Bacc("TRN2", target_bir_lowering=False, debug=False,
                   num_devices=n_cores)
    in_specs = [
        ("xn_q", [T, DIM]), ("xn_k", [T, DIM]), ("xn_v", [T, DIM]),
        ("xT_q", [128, NT * DIM]), ("xT_k", [128, NT * DIM]),
        ("xT_v", [128, NT * DIM]),
        ("Wp", [128, 8 * DIM]), ("WoT", [128, 8 * DIM]),
        ("negu", [1, DIM]), ("vrow", [1, DIM]), ("bout", [1, DIM]),
        ("ones", [128, 128]), ("ident", [128, 128]), ("mask", [128, 1024]),
        ("wp1T", [128, 256]), ("wp2T", [128, 3]), ("b1row", [1, 128]),
        ("gbc", [8, 128]), ("bbc", [8, 128]), ("b2bc", [8, 3]),
    ]
    in_aps = [nc.dram_tensor(n, s, F32, kind="ExternalInput").ap()
              for n, s in in_specs]
    y_ap = nc.dram_tensor("y", [T, DIM], F32, kind="ExternalOutput").ap()
    with tile.TileContext(nc) as tc:
        attn_kernel(tc, [y_ap], in_aps, n_cores=n_cores)
    nc.compile()
    _BUILT[n_cores] = nc
    return nc


def kernel(q, k, v, ln_g, ln_b, w_in, wp_w1, wp_b1, wp_ln_g, wp_ln_b,
           wp_w2, wp_b2, w_out, b_out):
    q = np.asarray(q, dtype=np.float32)
    k = np.asarray(k, dtype=np.float32)
    v = np.asarray(v, dtype=np.float32)
    ln_g = np.asarray(ln_g, np.float32); ln_b = np.asarray(ln_b, np.float32)
    w_in = np.asarray(w_in, np.float32); w_out = np.asarray(w_out, np.float32)
    b_out = np.asarray(b_out, np.float32)
    wp_w1 = np.asarray(wp_w1, np.float32); wp_b1 = np.asarray(wp_b1, np.float32)
    wp_ln_g = np.asarray(wp_ln_g, np.float32)
    wp_ln_b = np.asarray(wp_ln_b, np.float32)
    wp_w2 = np.asarray(wp_w2, np.float32); wp_b2 = np.asarray(wp_b2, np.float32)

    # host weight prep (folded layernorm)
    W = w_in.T                                     # [DIM, HD]
    Wp = (ln_g[:, None] * W)
    negu = -(ln_g @ W)[None, :]
    vrow = (ln_b @ W)[None, :]
    Wp_t = np.ascontiguousarray(
        Wp.reshape(8, 128, 2, 512).transpose(1, 0, 2, 3)).reshape(128, -1)
    WoT = np.ascontiguousarray(
        w_out.T.reshape(8, 128, DIM).transpose(1, 0, 2)).reshape(128, -1)
    shared = {
        "Wp": Wp_t, "WoT": WoT, "negu": negu, "vrow": vrow,
        "bout": b_out[None, :],
        "ones": np.ones((128, 128), np.float32),
        "ident": np.eye(128, dtype=np.float32),
        "mask": np.tile((1.0 - np.eye(128)).astype(np.float32), (1, 8)),
        "wp1T": np.ascontiguousarray(wp_w1.T.reshape(2, 128, 128)
                                     .transpose(1, 0, 2)).reshape(128, 256),
        "wp2T": np.ascontiguousarray(wp_w2.T),
        "b1row": wp_b1[None, :],
        "gbc": np.tile(wp_ln_g[None, :], (8, 1)),
        "bbc": np.tile(wp_ln_b[None, :], (8, 1)),
        "b2bc": np.tile(wp_b2[None, :], (8, 1)),
    }
    shared = {kk: np.ascontiguousarray(vv, np.float32)
              for kk, vv in shared.items()}

    qf = q.reshape(QB * N, DIM)
    kf = k.reshape(QB * N, DIM)
    vf = v.reshape(QB * N, DIM)
    in_maps = []
    for c in range(N_CORES):
        sl = slice(c * T, (c + 1) * T)
        m = dict(shared)
        for nm, arr in (("q", qf[sl]), ("k", kf[sl]), ("v", vf[sl])):
            m[f"xn_{nm}"] = np.ascontiguousarray(arr)
            m[f"xT_{nm}"] = np.ascontiguousarray(
                arr.reshape(NT, 128, 8, 128).transpose(3, 0, 2, 1)
            ).reshape(128, NT * DIM)
        in_maps.append(m)

    nc = _build()
    res = bass_utils.run_bass_kernel_spmd(nc, in_maps,
                                          core_ids=list(range(N_CORES)))
    global LAST_RESULTS
    LAST_RESULTS = res
    out = np.concatenate([r["y"] for r in res.results], axis=0)
    return out.reshape(QB, N, DIM)


LAST_RESULTS = None



# revision 3
# speedup vs baseline: 1.0113x; 1.0113x over previous
"""Trainium2 Bass kernel for nn_Attention_9096740733536 (sparse_attention).

Sharding: data-parallel over the QB (task) dim across 8 cores (2 tasks/core),
one mid-kernel AllReduce of [feat_corr partials | q_global | k_global] sums
(bf16 payload). The attention math is algebraically collapsed: mixed scores
are linear (no softmax), so
  out[h,q] = alpha_h*(Fq/qn) @ ((Fk/kn)^T @ Fv) + ww_h*q_ratio (x) (k_ratio^T Fv)
with 128x128 inner matrices instead of 512x512 score matrices. LayerNorm's
mean-correction is folded into the projection weights on the host (column
centering); the 1/sigma scale is applied at PSUM eviction. All PE operands
are bf16 (fp32 PSUM accumulation). Per-token/head stats come from batched
bn_stats (DVE); centering and the Fv/kn scale run on GPSIMD to keep ACT/DVE
off the critical path. Everything except the alpha/ww scaling runs before
the AllReduce; alpha is folded into the o1 eviction and ww/alpha into the
mv rows, so the post-collective tail is just o1 + output projection.
"""
import numpy as np
from contextlib import ExitStack

import concourse.bass as bass
import concourse.tile as tile
from concourse import bacc, mybir
from concourse import bass_utils
from concourse._compat import with_exitstack

F32 = mybir.dt.float32
BF16 = mybir.dt.bfloat16
AF = mybir.ActivationFunctionType
ALU = mybir.AluOpType
AX = mybir.AxisListType

H, D, DIM = 8, 128, 1024
QB, N = 16, 512
N_CORES = 8
T = QB * N // N_CORES          # 1024 tokens per core
NT = T // 128                  # 8 token tiles per core
NTASK = T // N                 # 2 tasks per core
LN_EPS = 1e-5
TOK_ALL = float(QB * N)


@with_exitstack
def attn_kernel(ctx: ExitStack, tc: tile.TileContext, outs, ins,
                n_cores=N_CORES, has_bias=False):
    nc = tc.nc
    y = outs[0]
    (xn_q, xn_k, xn_v, xT_q, xT_k, xT_v, Wp_d, WoT_d, vrow_d, bout_d,
     ones_d, onesbf_d, identbf_d, ident_d, mask_d, wp1T_d, wp2T_d, b1_d,
     gbc_d, bbc_d, b2bc_d) = ins

    consts = ctx.enter_context(tc.tile_pool(name="consts", bufs=1))
    fpool = ctx.enter_context(tc.tile_pool(name="fpool", bufs=1))
    stat1 = ctx.enter_context(tc.tile_pool(name="stat1", bufs=1))
    dram = ctx.enter_context(tc.tile_pool(name="dram", bufs=1, space="DRAM"))
    attn = ctx.enter_context(tc.tile_pool(name="attn", bufs=1))
    late = ctx.enter_context(tc.tile_pool(name="late", bufs=1))
    qcpool = ctx.enter_context(tc.tile_pool(name="qcpool", bufs=64))

    # PSUM banks: phase1 = p1(3)+trp(2)+fc(2)+gk(1); post-p1 the p1 pool
    # frees and ps_small(1) enters; phase 4b/5 run on o1(3)+p5(2).
    pre = ExitStack()
    ps_trp = pre.enter_context(tc.tile_pool(name="ps_trp", bufs=2,
                                            space="PSUM"))
    ps_fc = pre.enter_context(tc.tile_pool(name="ps_fc", bufs=2,
                                           space="PSUM"))
    ps_gk = pre.enter_context(tc.tile_pool(name="ps_gk", bufs=1,
                                           space="PSUM"))
    trsc = pre.enter_context(tc.tile_pool(name="trsc", bufs=2))

    # ---- constants needed inside phase 1 ----
    eps = consts.tile([128, 1], F32)
    nc.vector.memset(eps[:], LN_EPS)
    if has_bias:
        vrow = consts.tile([1, DIM], BF16)
        nc.sync.dma_start(vrow[:], vrow_d[:])
        bout = consts.tile([1, DIM], BF16)
        nc.sync.dma_start(bout[:], bout_d[:])
        onebf_row = consts.tile([1, 128], BF16)
        nc.vector.memset(onebf_row[:], 1.0)

    # ---- persistent F tensors: [128 tok, t*1024 + h*128 + d], bf16 ----
    Fq = fpool.tile([128, NT * DIM], BF16)
    Fk = fpool.tile([128, NT * DIM], BF16)
    Fv = fpool.tile([128, NT * DIM], BF16)

    xns = [xn_q, xn_k, xn_v]
    xTs = [xT_q, xT_k, xT_v]
    Fs = [Fq, Fk, Fv]

    # ---- per-(tile,head) stats: qa/ka hold (mean, var) pairs, col 2c/2c+1
    # for c = t*8+h; dense derived tiles are indexed by c ----
    qa = stat1.tile([128, 128], F32)
    ka = stat1.tile([128, 128], F32)
    qmean = stat1.tile([128, 64], F32)      # NEGATED mean (ACT/Pool bias)
    qninv = stat1.tile([128, 64], F32)
    kninv = stat1.tile([128, 64], F32)
    kn = stat1.tile([128, 64], F32)
    qrb = stat1.tile([128, 64], BF16)       # q_ratio (PE transpose input)
    krkn = stat1.tile([128, 64], BF16)      # k_ratio*kn (PE lhsT)
    kr = stat1.tile([128, 64], F32)
    rscr = stat1.tile([128, 128], F32)

    def derived(a, ninv, ratio, jh, n_out=None, negmean=None):
        # a: (m,v) pairs; group c in [32jh, 32jh+32)
        m = a[:, 64 * jh: 64 * jh + 64: 2]
        v = a[:, 64 * jh + 1: 64 * jh + 64: 2]
        sl = slice(32 * jh, 32 * jh + 32)
        t1 = rscr[:, 0:32]
        t2 = rscr[:, 32:64]
        t3 = rscr[:, 64:96]
        # qn = sqrt(D*(m^2 + v)) ; ninv = 1/qn
        nc.vector.tensor_tensor(t1, m, m, op=ALU.mult)
        nc.vector.tensor_tensor(t1, t1, v, op=ALU.add)
        if n_out is not None:
            nc.scalar.activation(n_out[:, sl], t1, AF.Sqrt, scale=float(D))
            nc.vector.reciprocal(ninv[:, sl], n_out[:, sl])
        else:
            nc.scalar.activation(ninv[:, sl], t1, AF.Sqrt, scale=float(D))
            nc.vector.reciprocal(ninv[:, sl], ninv[:, sl])
        # unbiased var vu = v*D/(D-1); ratio = 2*min(vu,1)/(vu+1)
        nc.vector.tensor_scalar_mul(t2, v, float(D) / (D - 1))
        nc.vector.tensor_scalar(t1, t2, 1.0, 2.0, ALU.min, ALU.mult)
        nc.vector.tensor_scalar_add(t3, t2, 1.0)
        nc.vector.reciprocal(t3, t3)
        nc.vector.tensor_tensor(ratio[:, sl], t1, t3, op=ALU.mult)
        if negmean is not None:
            nc.vector.tensor_scalar_mul(negmean[:, sl], m, -1.0)

    identbf = consts.tile([128, 128], BF16)
    nc.scalar.dma_start(identbf[:], identbf_d[:])
    onesbf = consts.tile([128, 8], BF16)
    nc.scalar.dma_start(onesbf[:], onesbf_d[:])

    # ======== Phase 1 (+ per-tile stats emission) ========
    qc_tiles = {}
    ar_in_g = dram.tile([128, 16], BF16)
    ar_out_g = dram.tile([128, 16], BF16)
    ar_in_fc = dram.tile([128, H * 128], BF16)
    ar_out_fc = dram.tile([128, H * 128], BF16)
    gk_ps = ps_gk.tile([128, 16], F32, tag="gk")
    arg = late.tile([128, 16], BF16)
    ar = late.tile([128, H * 128], BF16)
    mm_raw = {}
    mv_raw = {}
    fqT_tiles = {}
    wqr_tiles = {}

    def emit_gk(j):
        # one accumulation group spans both tasks (opened at t=0's chunk,
        # closed by the post-phase-1 chunk)
        for t in range(4 * j, 4 * j + 4):
            for h in range(H):
                sl = slice(t * DIM + h * 128, t * DIM + h * 128 + 128)
                first = (j == 0 and t == 0 and h == 0)
                last = (j == 1 and t == NT - 1 and h == H - 1)
                nc.tensor.matmul(gk_ps[:, h:h + 1],
                                 Fq[:, sl], onesbf[:, 0:1],
                                 start=first, stop=last,
                                 skip_group_check=True)
                nc.tensor.matmul(gk_ps[:, 8 + h:9 + h],
                                 Fk[:, sl], onesbf[:, 0:1],
                                 start=False, stop=False,
                                 skip_group_check=True)

    def emit_fqscale(j):
        # in-place Fq <- Fq/qn (after gk + qc reads of raw Fq)
        for h in range(H):
            for t in range(4 * j, 4 * j + 4):
                sl = slice(t * DIM + h * 128, t * DIM + h * 128 + 128)
                c = slice(t * 8 + h, t * 8 + h + 1)
                nc.vector.tensor_scalar(Fq[:, sl], Fq[:, sl], qninv[:, c],
                                        None, ALU.mult)

    def emit_mmv(j, heads=None):
        for h in (range(H) if heads is None else heads):
            mm_ps = ps_fc.tile([128, 128], F32, tag="fc128", name="mm_ps")
            trp_f32 = ps_trp.tile([128, 512], F32, tag="trp", name="trp_ps")
            mv_ps = trp_f32[0:1, 320:448]
            for ti in range(4):
                t = 4 * j + ti
                sl = slice(t * DIM + h * 128, t * DIM + h * 128 + 128)
                nc.tensor.matmul(mm_ps[:], Fk[:, sl], Fv[:, sl],
                                 start=(ti == 0), stop=(ti == 3))
                nc.tensor.matmul(mv_ps[:],
                                 krkn[:, t * 8 + h:t * 8 + h + 1],
                                 Fv[:, sl], start=(ti == 0), stop=(ti == 3))
            mm = attn.tile([128, 128], BF16, tag=f"mm{h}{j}", name="mm")
            nc.vector.tensor_copy(mm[:], mm_ps[:])
            mv = attn.tile([1, 128], BF16, tag=f"mv{h}{j}", name="mv")
            nc.vector.tensor_copy(mv[:], mv_ps[:])
            mm_raw[(h, j)] = mm
            mv_raw[(h, j)] = mv

    def emit_trp(j, heads=None):
        for h in (range(H) if heads is None else heads):
            trp_f32 = ps_trp.tile([128, 512], F32, tag="trp", name="trp_ps")
            trp_bf = trp_f32.bitcast(BF16)
            for ti in range(4):
                t = 4 * j + ti
                sl = slice(t * DIM + h * 128, t * DIM + h * 128 + 128)
                nc.tensor.transpose(trp_bf[:, ti * 128:(ti + 1) * 128],
                                    Fq[:, sl], identbf[:])
            c0 = 4 * j * 8 + h
            wq_ps = trp_bf[0:4, 512:640]
            nc.tensor.transpose(wq_ps, qrb[:, c0:c0 + 25:8], identbf[:])
            fqTs = attn.tile([128, 512], BF16, tag=f"fqT{h}{j}", name="fqTs")
            nc.scalar.copy(fqTs[:, 0:256], trp_bf[:, 0:256])
            nc.vector.tensor_copy(fqTs[:, 256:512], trp_bf[:, 256:512])
            wq4 = trsc.tile([4, 128], BF16, tag="wq4", name="wq4")
            nc.scalar.copy(wq4[:], wq_ps)
            wqr = attn.tile([1, 512], BF16, tag=f"wqr{h}{j}", name="wqr")
            nc.scalar.dma_start(wqr[:], wq4[:])
            fqT_tiles[(h, j)] = fqTs
            wqr_tiles[(h, j)] = wqr

    with tc.tile_pool(name="ph1", bufs=1) as ph1, \
         tc.tile_pool(name="xpool", bufs=3) as xpool, \
         tc.tile_pool(name="spool", bufs=3) as spool, \
         tc.tile_pool(name="bnpool", bufs=2) as bnpool, \
         tc.tile_pool(name="ps_p1", bufs=3, space="PSUM") as ps_p1:
        Wp = ph1.tile([128, 8 * DIM], BF16)

        def wp_chunk(ss, q):
            q.dma_start(Wp[:, ss * DIM:(ss + 1) * DIM],
                        Wp_d[:, ss * DIM:(ss + 1) * DIM])

        x0 = {}
        xT0s = [xpool.tile([128, DIM], BF16, tag="xT", name=f"xT0_{i}")
                for i in range(3)]
        xn0s = [xpool.tile([128, DIM], BF16, tag="xn", name=f"xn0_{i}")
                for i in range(3)]
        # sync: Wp0, xTq, Wp3, Wp6, xTk ; scalar: Wp1, Wp4, xn*, Wp7 ;
        # gpsimd: Wp2, xTv, Wp5  (first group's chunks arrive in MM order)
        wp_chunk(0, nc.sync)
        wp_chunk(1, nc.scalar)
        wp_chunk(2, nc.gpsimd)
        nc.sync.dma_start(xT0s[0][:, 0:512], xTs[0][:, 0:512])
        nc.sync.dma_start(xT0s[0][:, 512:1024], xTs[0][:, 512:1024])
        wp_chunk(3, nc.sync)
        wp_chunk(4, nc.scalar)
        wp_chunk(5, nc.gpsimd)
        nc.gpsimd.dma_start(xT0s[2][:, 0:512], xTs[2][:, 0:512])
        nc.gpsimd.dma_start(xT0s[2][:, 512:1024], xTs[2][:, 512:1024])
        wp_chunk(6, nc.sync)
        wp_chunk(7, nc.scalar)
        nc.sync.dma_start(xT0s[1][:, 0:512], xTs[1][:, 0:512])
        nc.sync.dma_start(xT0s[1][:, 512:1024], xTs[1][:, 512:1024])
        for i in range(3):
            nc.scalar.dma_start(xn0s[i][:], xns[i][0:128, :])
            x0[i] = (xT0s[i], xn0s[i])
        WoT = late.tile([128, 8 * DIM], BF16)
        for t in range(NT):
            st = spool.tile([128, 6], F32, tag="st")
            bn6 = spool.tile([128, 36], F32, tag="bn6")
            sg = spool.tile([128, 3], F32, tag="sg")
            rsig = spool.tile([128, 3], F32, tag="rsig")
            bnq = bnpool.tile([128, 48], F32, tag="bnq")
            bnk = bnpool.tile([128, 48], F32, tag="bnk")
            for i in range(3):
                # LN stats for this tensor only -> per-tensor rsig, so each
                # tensor's evictions gate only on its own xn load
                if t == 0:
                    xn = x0[i][1]
                else:
                    xn = xpool.tile([128, DIM], BF16, tag="xn")
                    nc.scalar.dma_start(xn[:],
                                        xns[i][t * 128:(t + 1) * 128, :])
                nc.vector.bn_stats(bn6[:, i * 12:i * 12 + 6], xn[:, 0:512])
                nc.vector.bn_stats(bn6[:, i * 12 + 6:i * 12 + 12],
                                   xn[:, 512:1024])
                nc.vector.bn_aggr(st[:, 2 * i:2 * i + 2],
                                  bn6[:, i * 12:i * 12 + 12])
                nc.scalar.activation(sg[:, i:i + 1],
                                     st[:, 2 * i + 1:2 * i + 2],
                                     AF.Sqrt, bias=eps[:])
                nc.vector.reciprocal(rsig[:, i:i + 1], sg[:, i:i + 1])
                if t == 0:
                    xT_t = x0[i][0]
                else:
                    xT_t = xpool.tile([128, DIM], BF16, tag="xT")
                    qa_, qb_ = ((nc.sync, nc.sync), (nc.sync, nc.sync),
                                (nc.gpsimd, nc.gpsimd))[i]
                    qa_.dma_start(xT_t[:, 0:512],
                                  xTs[i][:, t * DIM:t * DIM + 512])
                    qb_.dma_start(xT_t[:, 512:1024],
                                  xTs[i][:, t * DIM + 512:(t + 1) * DIM])
                for half in range(2):
                    o = half * 512
                    acc = ps_p1.tile([128, 512], F32, tag="p1")
                    for s in range(8):
                        nc.tensor.matmul(
                            acc[:], xT_t[:, s * 128:(s + 1) * 128],
                            Wp[:, s * DIM + o: s * DIM + o + 512],
                            start=(s == 0),
                            stop=(s == 7 and not has_bias))
                    if has_bias:
                        nc.tensor.matmul(acc[:], onebf_row[:],
                                         vrow[:, o:o + 512],
                                         start=False, stop=True)
                    dst = Fs[i][:, t * DIM + o: t * DIM + o + 512]
                    nc.scalar.mul(dst, acc[:], rsig[:, i:i + 1])
                # F stats as soon as this tensor's tiles land (tail latency
                # of the last tile gates feat_corr / the M stage)
                if i == 0:
                    for h in range(H):
                        nc.vector.bn_stats(
                            bnq[:, h * 6:h * 6 + 6],
                            Fq[:, t * DIM + h * 128:t * DIM + h * 128 + 128])
                    for h in range(H):
                        c = t * 8 + h
                        nc.vector.bn_aggr(qa[:, 2 * c:2 * c + 2],
                                          bnq[:, h * 6:h * 6 + 6])
                    nc.vector.tensor_scalar_mul(qmean[:, t * 8:t * 8 + 8],
                                                qa[:, 16 * t:16 * t + 16:2],
                                                -1.0)
                    for h in range(H):
                        c = t * 8 + h
                        fsl = slice(t * DIM + h * 128,
                                    t * DIM + h * 128 + 128)
                        qc = qcpool.tile([128, 128], BF16, tag="qc",
                                         name="qc")
                        nc.gpsimd.tensor_scalar(qc[:], Fq[:, fsl],
                                                qmean[:, c:c + 1], None,
                                                ALU.add)
                        qc_tiles[(t, h)] = qc
                elif i == 1:
                    for h in range(H):
                        nc.vector.bn_stats(
                            bnk[:, h * 6:h * 6 + 6],
                            Fk[:, t * DIM + h * 128:t * DIM + h * 128 + 128])
                    for h in range(H):
                        c = t * 8 + h
                        nc.vector.bn_aggr(ka[:, 2 * c:2 * c + 2],
                                          bnk[:, h * 6:h * 6 + 6])
            nc.scalar.dma_start(WoT[:, t * DIM:(t + 1) * DIM],
                                WoT_d[:, t * DIM:(t + 1) * DIM])

            if t in (3, 7):
                jh = t // 4
                sl = slice(32 * jh, 32 * jh + 32)
                derived(qa, qninv, qrb, jh)
                derived(ka, kninv, kr, jh, n_out=kn)
                nc.vector.tensor_tensor(krkn[:, sl], kr[:, sl], kn[:, sl],
                                        op=ALU.mult)
                for tt in range(4 * jh, 4 * jh + 4):
                    for h in range(H):
                        c = tt * 8 + h
                        fsl = slice(tt * DIM + h * 128,
                                    tt * DIM + h * 128 + 128)
                        nc.vector.tensor_scalar(
                            Fv[:, fsl], Fv[:, fsl],
                            kninv[:, c:c + 1], None, ALU.mult)
            # task-0 attention-core work rides inside phase 1: its PE bursts
            # interleave with the dense projection stream and its evictions
            # land on engine slack
            if t == 5:
                emit_gk(0)
                emit_fqscale(0)
                emit_mmv(0)
            elif t == 6:
                emit_trp(0)

    # ---- remaining constants (first used after phase 1) ----
    ident8 = consts.tile([8, 8], F32)
    nc.sync.dma_start(ident8[:], ident_d[0:8, 0:8])
    ones = consts.tile([128, 8], F32)
    nc.sync.dma_start(ones[:], ones_d[:, 0:8])
    ones8 = consts.tile([1, 8], F32)
    nc.sync.dma_start(ones8[:], ones_d[0:1, 0:8])
    mask_nd = consts.tile([128, H * 128], BF16)
    nc.scalar.dma_start(mask_nd[:], mask_d[:])
    wp1T = consts.tile([128, 256], F32)
    nc.scalar.dma_start(wp1T[:], wp1T_d[:])
    wp2T = consts.tile([128, 3], F32)
    nc.scalar.dma_start(wp2T[:], wp2T_d[:])
    b1row = consts.tile([1, 128], F32)
    nc.scalar.dma_start(b1row[:], b1_d[:])
    gbc = consts.tile([8, 128], F32)
    nc.scalar.dma_start(gbc[:], gbc_d[:])
    bbc = consts.tile([8, 128], F32)
    nc.scalar.dma_start(bbc[:], bbc_d[:])
    b2bc = consts.tile([8, 3], F32)
    nc.scalar.dma_start(b2bc[:], b2bc_d[:])

    # ======== post-phase-1: task-1 region + collectives ========
    ps_small = pre.enter_context(tc.tile_pool(name="ps_small", bufs=1,
                                              space="PSUM"))
    emit_gk(1)
    gk_sb = trsc.tile([128, 16], BF16, tag="gksb", name="gk_sb")
    nc.scalar.copy(gk_sb[:], gk_ps[:])
    nc.sync.dma_start(ar_in_g[:], gk_sb[:])
    if n_cores > 1:
        nc.gpsimd.collective_compute(
            "AllReduce", ALU.add,
            replica_groups=[list(range(n_cores))],
            ins=[ar_in_g.opt()], outs=[ar_out_g.opt()])
    else:
        nc.sync.dma_start(ar_out_g[:], ar_in_g[:])
    arg = late.tile([128, 16], BF16)
    nc.scalar.dma_start(arg[:], ar_out_g[:])

    # feat_corr partials per head, shipped as they complete
    for h in range(H):
        fc_ps = ps_fc.tile([128, 128], F32, tag="fc128", name="fc_ps")
        for t in range(NT):
            nc.tensor.matmul(fc_ps[:], qc_tiles[(t, h)][:],
                             qc_tiles[(t, h)][:],
                             start=(t == 0), stop=(t == NT - 1))
        fc_sb = trsc.tile([128, 128], BF16, tag="fcsb", name="fc_sb")
        nc.scalar.copy(fc_sb[:], fc_ps[:])
        nc.sync.dma_start(ar_in_fc[:, h * 128:(h + 1) * 128], fc_sb[:])
    if n_cores > 1:
        nc.gpsimd.collective_compute(
            "AllReduce", ALU.add,
            replica_groups=[list(range(n_cores))],
            ins=[ar_in_fc.opt()], outs=[ar_out_fc.opt()])
    else:
        nc.sync.dma_start(ar_out_fc[:], ar_in_fc[:])
    ar = late.tile([128, H * 128], BF16)
    nc.sync.dma_start(ar[:], ar_out_fc[:])

    emit_fqscale(1)
    emit_mmv(1)

    # ======== Phase 3a: weight predictor (needs only gk slice) ========
    featsq = stat1.tile([128, 8], F32)
    nc.vector.tensor_scalar_mul(featsq[:], arg[:, 0:8], 1.0 / TOK_ALL)
    featsk = stat1.tile([128, 8], F32)
    nc.vector.tensor_scalar_mul(featsk[:], arg[:, 8:16], 1.0 / TOK_ALL)
    h1_ps = ps_small.tile([8, 128], F32, tag="sm", name="h1_ps")
    nc.tensor.matmul(h1_ps[:], featsq[:], wp1T[:, 0:128], start=True,
                     stop=False)
    nc.tensor.matmul(h1_ps[:], featsk[:], wp1T[:, 128:256], start=False,
                     stop=False)
    nc.tensor.matmul(h1_ps[:], ones8[:], b1row[:], start=False, stop=True)
    h1 = stat1.tile([8, 128], F32)
    nc.scalar.copy(h1[:], h1_ps[:])
    w_mu = stat1.tile([8, 4], F32)
    sq8 = stat1.tile([8, 128], F32)
    nc.vector.reduce_sum(w_mu[:, 0:1], h1[:], axis=AX.X)
    nc.vector.tensor_scalar_mul(w_mu[:, 0:1], w_mu[:, 0:1], 1.0 / D)
    nc.scalar.activation(sq8[:], h1[:], AF.Square, accum_out=w_mu[:, 1:2])
    nc.vector.tensor_scalar_mul(w_mu[:, 1:2], w_mu[:, 1:2], 1.0 / D)
    nc.vector.tensor_tensor(w_mu[:, 2:3], w_mu[:, 0:1], w_mu[:, 0:1],
                            op=ALU.mult)
    nc.vector.tensor_tensor(w_mu[:, 2:3], w_mu[:, 1:2], w_mu[:, 2:3],
                            op=ALU.subtract)
    nc.scalar.activation(w_mu[:, 3:4], w_mu[:, 2:3], AF.Sqrt, bias=eps[0:8, :])
    nc.vector.reciprocal(w_mu[:, 3:4], w_mu[:, 3:4])
    h1n = stat1.tile([8, 128], F32)
    nc.vector.tensor_scalar(h1n[:], h1[:], w_mu[:, 0:1], w_mu[:, 3:4],
                            ALU.subtract, ALU.mult)
    nc.vector.tensor_tensor(h1n[:], h1n[:], gbc[:], op=ALU.mult)
    nc.vector.tensor_tensor(h1n[:], h1n[:], bbc[:], op=ALU.add)
    nc.vector.tensor_scalar_max(h1n[:], h1n[:], 0.0)
    h1T_ps = ps_small.tile([128, 8], F32, tag="sm", name="h1T_ps")
    nc.tensor.transpose(h1T_ps[:], h1n[:], ident8[:])
    h1T = stat1.tile([128, 8], F32)
    nc.scalar.copy(h1T[:], h1T_ps[:])
    lg_ps = ps_small.tile([8, 3], F32, tag="sm", name="lg_ps")
    nc.tensor.matmul(lg_ps[:], h1T[:], wp2T[:], start=True, stop=True)
    lg = stat1.tile([8, 8], F32)
    nc.scalar.copy(lg[:, 0:3], lg_ps[:])
    nc.vector.tensor_tensor(lg[:, 0:3], lg[:, 0:3], b2bc[:], op=ALU.add)
    # logits are O(1): skip the (mathematically redundant) max-subtraction
    nc.scalar.activation(lg[:, 0:3], lg[:, 0:3], AF.Exp)
    nc.vector.reduce_sum(lg[:, 4:5], lg[:, 0:3], axis=AX.X)
    nc.vector.reciprocal(lg[:, 4:5], lg[:, 4:5])
    nc.vector.tensor_scalar(lg[:, 0:3], lg[:, 0:3], lg[:, 4:5], None,
                            ALU.mult)



    # ======== Phase 3b: decorr scale (needs feat_corr block) ========
    ssq = stat1.tile([128, 8], F32)
    msk = late.tile([128, H * 128], BF16)
    nc.vector.tensor_tensor(msk[:], ar[:], mask_nd[:], op=ALU.mult)
    sqf = late.tile([128, H * 128], F32)
    nc.scalar.activation(sqf[:], msk[:], AF.Square)
    nc.vector.reduce_sum(ssq[:],
                         sqf[:].rearrange("p (h d) -> p h d", h=8),
                         axis=AX.X)
    ss_ps = ps_small.tile([8, 8], F32, tag="sm", name="ss_ps")
    nc.tensor.matmul(ss_ps[:], ssq[:], ones[:, 0:8], start=True, stop=True)
    dsc = stat1.tile([8, 8], F32)
    nc.scalar.activation(dsc[:, 0:1], ss_ps[0:8, 0:1], AF.Sqrt)
    nc.scalar.activation(dsc[:, 1:2], dsc[:, 0:1], AF.Exp,
                         scale=-5.0 / (D * D * TOK_ALL))

    # alpha = w0 + w1*dsc ; wsc = w2/alpha ; flat row [alpha(8) | wsc(8)]
    aw = stat1.tile([8, 4], F32)
    nc.vector.tensor_tensor(aw[:, 0:1], lg[:, 1:2], dsc[:, 1:2], op=ALU.mult)
    nc.vector.tensor_tensor(aw[:, 0:1], aw[:, 0:1], lg[:, 0:1], op=ALU.add)
    nc.vector.reciprocal(aw[:, 2:3], aw[:, 0:1])
    nc.vector.tensor_tensor(aw[:, 1:2], lg[:, 2:3], aw[:, 2:3], op=ALU.mult)
    awT_ps = ps_small.tile([1, 8], F32, tag="sm", name="awT_ps")
    nc.tensor.transpose(awT_ps[:], aw[:, 0:1], ident8[:])
    awTa = stat1.tile([1, 8], F32)
    nc.scalar.copy(awTa[:], awT_ps[:])
    awT_ps2 = ps_small.tile([1, 8], F32, tag="sm", name="awT_ps2")
    nc.tensor.transpose(awT_ps2[:], aw[:, 1:2], ident8[:])
    awTb = stat1.tile([1, 8], F32)
    nc.scalar.copy(awTb[:], awT_ps2[:])
    abc = stat1.tile([128, 8], F32)
    nc.gpsimd.partition_broadcast(abc[:], awTa[:])
    wscbc = stat1.tile([128, 8], F32)
    nc.gpsimd.partition_broadcast(wscbc[:], awTb[:])
    pre.close()

    # ======== Phase 4b + 5: scaled attention + output projection ========
    with tc.tile_pool(name="ph4", bufs=2) as ph4, \
         tc.tile_pool(name="o1pool", bufs=10) as o1pool, \
         tc.tile_pool(name="ps_o1", bufs=3, space="PSUM") as ps_o1, \
         tc.tile_pool(name="ps_p5", bufs=2, space="PSUM") as ps_p5:
        o1_tiles = {}
        for j in range(NTASK):
            for h in range(H):
                # mv row scaled by ww/alpha (tiny); alpha applied at eviction
                mvw = ph4.tile([1, 128], BF16, tag="mvw", name="mvw")
                nc.vector.tensor_scalar(mvw[:], mv_raw[(h, j)][:],
                                        wscbc[0:1, h:h + 1], None,
                                        ALU.mult)
                o1_ps = ps_o1.tile([128, 512], F32, tag="o1", name="o1_ps")
                nc.tensor.matmul(o1_ps[:], mm_raw[(h, j)][:],
                                 fqT_tiles[(h, j)][:], start=True, stop=False)
                nc.tensor.matmul(o1_ps[:], mvw[:], wqr_tiles[(h, j)][:],
                                 start=False, stop=True)
                o1 = o1pool.tile([128, 512], BF16, tag="o1sb", name="o1_sb")
                if h % 2 == 0:
                    nc.vector.tensor_scalar(o1[:], o1_ps[:], abc[:, h:h + 1],
                                            None, ALU.mult)
                else:
                    nc.scalar.mul(o1[:], o1_ps[:], abc[:, h:h + 1])
                o1_tiles[(h, j)] = o1

            # ---- output projection for this task ----
            for t in range(4 * j, 4 * j + 4):
                ti = t % 4
                for half in range(2):
                    o = half * 512
                    op_ps = ps_p5.tile([128, 512], F32, tag="p5",
                                       name="op_ps")
                    for h in range(H):
                        nc.tensor.matmul(
                            op_ps[:],
                            o1_tiles[(h, j)][:, ti * 128:(ti + 1) * 128],
                            WoT[:, h * DIM + o: h * DIM + o + 512],
                            start=(h == 0),
                            stop=(h == H - 1 and not has_bias))
                    if has_bias:
                        nc.tensor.matmul(op_ps[:], onebf_row[:],
                                         bout[:, o:o + 512],
                                         start=False, stop=True)
                    ysb = ph4.tile([128, 512], F32, tag="ysb", name="ysb")
                    if j == 1 and (t + half) % 2 == 1:
                        nc.vector.tensor_copy(ysb[:], op_ps[:])
                    else:
                        nc.scalar.copy(ysb[:], op_ps[:])
                    qy = nc.sync if (t + half) % 2 == 0 else nc.scalar
                    qy.dma_start(y[t * 128:(t + 1) * 128, o:o + 512],
                                 ysb[:])


_BUILT = {}


def _build(n_cores=N_CORES, has_bias=False):
    key = (n_cores, has_bias)
    if key in _BUILT:
        return _BUILT[key]
    nc = bacc.Bacc("TRN2", target_bir_lowering=False, debug=False,
                   num_devices=n_cores)
    in_specs = [
        ("xn_q", [T, DIM], BF16), ("xn_k", [T, DIM], BF16),
        ("xn_v", [T, DIM], BF16),
        ("xT_q", [128, NT * DIM], BF16), ("xT_k", [128, NT * DIM], BF16),
        ("xT_v", [128, NT * DIM], BF16),
        ("Wp", [128, 8 * DIM], BF16), ("WoT", [128, 8 * DIM], BF16),
        ("vrow", [1, DIM], BF16), ("bout", [1, DIM], BF16),
        ("ones", [128, 128], F32), ("onesbf", [128, 8], BF16),
        ("identbf", [128, 128], BF16), ("ident", [128, 128], F32),
        ("mask", [128, 1024], BF16),
        ("wp1T", [128, 256], F32), ("wp2T", [128, 3], F32),
        ("b1row", [1, 128], F32),
        ("gbc", [8, 128], F32), ("bbc", [8, 128], F32), ("b2bc", [8, 3], F32),
    ]
    in_aps = [nc.dram_tensor(n, s, dt, kind="ExternalInput").ap()
              for n, s, dt in in_specs]
    y_ap = nc.dram_tensor("y", [T, DIM], F32, kind="ExternalOutput").ap()
    with tile.TileContext(nc) as tc:
        attn_kernel(tc, [y_ap], in_aps, n_cores=n_cores, has_bias=has_bias)
    nc.compile()
    _BUILT[key] = nc
    return nc


def _bf(x):
    import ml_dtypes
    return np.asarray(x, dtype=ml_dtypes.bfloat16)


def kernel(q, k, v, ln_g, ln_b, w_in, wp_w1, wp_b1, wp_ln_g, wp_ln_b,
           wp_w2, wp_b2, w_out, b_out):
    q = np.asarray(q, dtype=np.float32)
    k = np.asarray(k, dtype=np.float32)
    v = np.asarray(v, dtype=np.float32)
    ln_g = np.asarray(ln_g, np.float32); ln_b = np.asarray(ln_b, np.float32)
    w_in = np.asarray(w_in, np.float32); w_out = np.asarray(w_out, np.float32)
    b_out = np.asarray(b_out, np.float32)
    wp_w1 = np.asarray(wp_w1, np.float32); wp_b1 = np.asarray(wp_b1, np.float32)
    wp_ln_g = np.asarray(wp_ln_g, np.float32)
    wp_ln_b = np.asarray(wp_ln_b, np.float32)
    wp_w2 = np.asarray(wp_w2, np.float32); wp_b2 = np.asarray(wp_b2, np.float32)

    # host weight prep: fold LN gain into W, then column-center so x @ Wp
    # carries the -mu*sum(g*W) correction implicitly
    W = w_in.T                                     # [DIM, HD]
    Wp = (ln_g[:, None] * W)
    Wp = Wp - Wp.mean(axis=0, keepdims=True)
    vrow = (ln_b @ W)[None, :]
    has_bias = bool(np.any(ln_b != 0.0) or np.any(b_out != 0.0))
    Wp_t = np.ascontiguousarray(
        Wp.reshape(8, 128, 2, 512).transpose(1, 0, 2, 3)).reshape(128, -1)
    WoT = np.ascontiguousarray(
        w_out.T.reshape(8, 128, DIM).transpose(1, 0, 2)).reshape(128, -1)
    shared = {
        "Wp": _bf(Wp_t), "WoT": _bf(WoT), "vrow": _bf(vrow),
        "bout": _bf(b_out[None, :]),
        "ones": np.ones((128, 128), np.float32),
        "onesbf": _bf(np.ones((128, 8), np.float32)),
        "identbf": _bf(np.eye(128, dtype=np.float32)),
        "ident": np.eye(128, dtype=np.float32),
        "mask": _bf(np.tile((1.0 - np.eye(128)).astype(np.float32), (1, 8))),
        "wp1T": np.ascontiguousarray(wp_w1.T.reshape(2, 128, 128)
                                     .transpose(1, 0, 2)).reshape(128, 256),
        "wp2T": np.ascontiguousarray(wp_w2.T),
        "b1row": wp_b1[None, :],
        "gbc": np.tile(wp_ln_g[None, :], (8, 1)),
        "bbc": np.tile(wp_ln_b[None, :], (8, 1)),
        "b2bc": np.tile(wp_b2[None, :], (8, 1)),
    }
    for kk in ("ones", "ident", "wp1T", "wp2T", "b1row", "gbc", "bbc",
               "b2bc"):
        shared[kk] = np.ascontiguousarray(shared[kk], np.float32)

    qf = q.reshape(QB * N, DIM)
    kf = k.reshape(QB * N, DIM)
    vf = v.reshape(QB * N, DIM)
    in_maps = []
    for c in range(N_CORES):
        sl = slice(c * T, (c + 1) * T)
        m = dict(shared)
        for nm, arr in (("q", qf[sl]), ("k", kf[sl]), ("v", vf[sl])):
            m[f"xn_{nm}"] = _bf(np.ascontiguousarray(arr))
            m[f"xT_{nm}"] = _bf(np.ascontiguousarray(
                arr.reshape(NT, 128, 8, 128).transpose(3, 0, 2, 1)
            ).reshape(128, NT * DIM))
        in_maps.append(m)

    nc = _build(has_bias=has_bias)
    res = bass_utils.run_bass_kernel_spmd(nc, in_maps,
                                          core_ids=list(range(N_CORES)))
    global LAST_RESULTS
    LAST_RESULTS = res
    out = np.concatenate([np.asarray(r["y"], np.float32)
                          for r in res.results], axis=0)
    return out.reshape(QB, N, DIM)


LAST_RESULTS = None


# revision 4
# speedup vs baseline: 1.0188x; 1.0074x over previous
"""Trainium2 Bass kernel for nn_Attention_9096740733536 (sparse_attention).

Sharding: data-parallel over the QB (task) dim across 8 cores (2 tasks/core),
one mid-kernel AllReduce of [feat_corr partials | q_global | k_global] sums
(bf16 payload). The attention math is algebraically collapsed: mixed scores
are linear (no softmax), so
  out[h,q] = alpha_h*(Fq/qn) @ ((Fk/kn)^T @ Fv) + ww_h*q_ratio (x) (k_ratio^T Fv)
with 128x128 inner matrices instead of 512x512 score matrices. LayerNorm's
mean-correction is folded into the projection weights on the host (column
centering); the 1/sigma scale is applied at PSUM eviction. All PE operands
are bf16 (fp32 PSUM accumulation). Per-token/head stats come from batched
bn_stats (DVE); centering and the Fv/kn scale run on GPSIMD to keep ACT/DVE
off the critical path. Everything except the alpha/ww scaling runs before
the AllReduce; alpha is folded into the o1 eviction and ww/alpha into the
mv rows, so the post-collective tail is just o1 + output projection.
"""
import numpy as np
from contextlib import ExitStack

import concourse.bass as bass
import concourse.tile as tile
from concourse import bacc, mybir
from concourse import bass_utils
from concourse._compat import with_exitstack

F32 = mybir.dt.float32
BF16 = mybir.dt.bfloat16
AF = mybir.ActivationFunctionType
ALU = mybir.AluOpType
AX = mybir.AxisListType

H, D, DIM = 8, 128, 1024
QB, N = 16, 512
N_CORES = 8
T = QB * N // N_CORES          # 1024 tokens per core
NT = T // 128                  # 8 token tiles per core
NTASK = T // N                 # 2 tasks per core
LN_EPS = 1e-5
TOK_ALL = float(QB * N)


@with_exitstack
def attn_kernel(ctx: ExitStack, tc: tile.TileContext, outs, ins,
                n_cores=N_CORES, has_bias=False):
    nc = tc.nc
    y = outs[0]
    (xn_q, xn_k, xn_v, xT_q, xT_k, xT_v, Wp_d, WoT_d, vrow_d, bout_d,
     ones_d, onesbf_d, identbf_d, ident_d, mask_d, wp1T_d, wp2T_d, b1_d,
     gbc_d, bbc_d, b2bc_d) = ins

    consts = ctx.enter_context(tc.tile_pool(name="consts", bufs=1))
    fpool = ctx.enter_context(tc.tile_pool(name="fpool", bufs=1))
    stat1 = ctx.enter_context(tc.tile_pool(name="stat1", bufs=1))
    dram = ctx.enter_context(tc.tile_pool(name="dram", bufs=1, space="DRAM"))
    attn = ctx.enter_context(tc.tile_pool(name="attn", bufs=1))
    late = ctx.enter_context(tc.tile_pool(name="late", bufs=1))
    qcpool = ctx.enter_context(tc.tile_pool(name="qcpool", bufs=64))

    # PSUM banks: phase1 = p1(3)+trp(2)+fc(2)+gk(1); post-p1 the p1 pool
    # frees and ps_small(1) enters; phase 4b/5 run on o1(3)+p5(2).
    pre = ExitStack()
    ps_trp = pre.enter_context(tc.tile_pool(name="ps_trp", bufs=2,
                                            space="PSUM"))
    ps_fc = pre.enter_context(tc.tile_pool(name="ps_fc", bufs=2,
                                           space="PSUM"))
    ps_gk = pre.enter_context(tc.tile_pool(name="ps_gk", bufs=1,
                                           space="PSUM"))
    trsc = pre.enter_context(tc.tile_pool(name="trsc", bufs=2))

    # ---- constants needed inside phase 1 ----
    eps = consts.tile([128, 1], F32)
    nc.vector.memset(eps[:], LN_EPS)
    if has_bias:
        vrow = consts.tile([1, DIM], BF16)
        nc.sync.dma_start(vrow[:], vrow_d[:])
        bout = consts.tile([1, DIM], BF16)
        nc.sync.dma_start(bout[:], bout_d[:])
        onebf_row = consts.tile([1, 128], BF16)
        nc.vector.memset(onebf_row[:], 1.0)

    # ---- persistent F tensors: [128 tok, t*1024 + h*128 + d], bf16 ----
    Fq = fpool.tile([128, NT * DIM], BF16)
    Fk = fpool.tile([128, NT * DIM], BF16)
    Fv = fpool.tile([128, NT * DIM], BF16)

    xns = [xn_q, xn_k, xn_v]
    xTs = [xT_q, xT_k, xT_v]
    Fs = [Fq, Fk, Fv]

    # ---- per-(tile,head) stats: qa/ka hold (mean, var) pairs, col 2c/2c+1
    # for c = t*8+h; dense derived tiles are indexed by c ----
    qa = stat1.tile([128, 128], F32)
    ka = stat1.tile([128, 128], F32)
    qmean = stat1.tile([128, 64], F32)      # NEGATED mean (ACT/Pool bias)
    qninv = stat1.tile([128, 64], F32)
    kninv = stat1.tile([128, 64], F32)
    kn = stat1.tile([128, 64], F32)
    qrb = stat1.tile([128, 64], BF16)       # q_ratio (PE transpose input)
    krkn = stat1.tile([128, 64], BF16)      # k_ratio*kn (PE lhsT)
    kr = stat1.tile([128, 64], F32)
    rscr = stat1.tile([128, 128], F32)

    def derived(a, ninv, ratio, jh, n_out=None, negmean=None):
        # a: (m,v) pairs; group c in [32jh, 32jh+32)
        m = a[:, 64 * jh: 64 * jh + 64: 2]
        v = a[:, 64 * jh + 1: 64 * jh + 64: 2]
        sl = slice(32 * jh, 32 * jh + 32)
        t1 = rscr[:, 0:32]
        t2 = rscr[:, 32:64]
        t3 = rscr[:, 64:96]
        # qn = sqrt(D*(m^2 + v)) ; ninv = 1/qn
        nc.vector.tensor_tensor(t1, m, m, op=ALU.mult)
        nc.vector.tensor_tensor(t1, t1, v, op=ALU.add)
        if n_out is not None:
            nc.scalar.activation(n_out[:, sl], t1, AF.Sqrt, scale=float(D))
            nc.vector.reciprocal(ninv[:, sl], n_out[:, sl])
        else:
            nc.scalar.activation(ninv[:, sl], t1, AF.Sqrt, scale=float(D))
            nc.vector.reciprocal(ninv[:, sl], ninv[:, sl])
        # unbiased var vu = v*D/(D-1); ratio = 2*min(vu,1)/(vu+1)
        nc.vector.tensor_scalar_mul(t2, v, float(D) / (D - 1))
        nc.vector.tensor_scalar(t1, t2, 1.0, 2.0, ALU.min, ALU.mult)
        nc.vector.tensor_scalar_add(t3, t2, 1.0)
        nc.vector.reciprocal(t3, t3)
        nc.vector.tensor_tensor(ratio[:, sl], t1, t3, op=ALU.mult)
        if negmean is not None:
            nc.vector.tensor_scalar_mul(negmean[:, sl], m, -1.0)

    identbf = consts.tile([128, 128], BF16)
    nc.scalar.dma_start(identbf[:], identbf_d[:])
    onesbf = consts.tile([128, 8], BF16)
    nc.scalar.dma_start(onesbf[:], onesbf_d[:])

    # ======== Phase 1 (+ per-tile stats emission) ========
    qc_tiles = {}
    ar_in_g = dram.tile([128, 16], BF16)
    ar_out_g = dram.tile([128, 16], BF16)
    ar_in_fc = dram.tile([128, H * 128], BF16)
    ar_out_fc = dram.tile([128, H * 128], BF16)
    gk_ps = ps_gk.tile([128, 16], F32, tag="gk")
    arg = late.tile([128, 16], BF16)
    ar = late.tile([128, H * 128], BF16)
    mm_raw = {}
    mv_raw = {}
    fqT_tiles = {}
    wqr_tiles = {}

    def emit_gk(j):
        # one accumulation group spans both tasks (opened at t=0's chunk,
        # closed by the post-phase-1 chunk)
        for t in range(4 * j, 4 * j + 4):
            for h in range(H):
                sl = slice(t * DIM + h * 128, t * DIM + h * 128 + 128)
                first = (j == 0 and t == 0 and h == 0)
                last = (j == 1 and t == NT - 1 and h == H - 1)
                nc.tensor.matmul(gk_ps[:, h:h + 1],
                                 Fq[:, sl], onesbf[:, 0:1],
                                 start=first, stop=last,
                                 skip_group_check=True)
                nc.tensor.matmul(gk_ps[:, 8 + h:9 + h],
                                 Fk[:, sl], onesbf[:, 0:1],
                                 start=False, stop=False,
                                 skip_group_check=True)

    def emit_fqscale(j):
        # in-place Fq <- Fq/qn (after gk + qc reads of raw Fq)
        for h in range(H):
            seng = nc.vector if h % 2 == 0 else nc.gpsimd
            for t in range(4 * j, 4 * j + 4):
                sl = slice(t * DIM + h * 128, t * DIM + h * 128 + 128)
                c = slice(t * 8 + h, t * 8 + h + 1)
                seng.tensor_scalar(Fq[:, sl], Fq[:, sl], qninv[:, c],
                                   None, ALU.mult)

    def emit_mmv(j, heads=None):
        for h in (range(H) if heads is None else heads):
            mm_ps = ps_fc.tile([128, 128], F32, tag="fc128", name="mm_ps")
            trp_f32 = ps_trp.tile([128, 512], F32, tag="trp", name="trp_ps")
            mv_ps = trp_f32[0:1, 320:448]
            for ti in range(4):
                t = 4 * j + ti
                sl = slice(t * DIM + h * 128, t * DIM + h * 128 + 128)
                nc.tensor.matmul(mm_ps[:], Fk[:, sl], Fv[:, sl],
                                 start=(ti == 0), stop=(ti == 3))
                nc.tensor.matmul(mv_ps[:],
                                 krkn[:, t * 8 + h:t * 8 + h + 1],
                                 Fv[:, sl], start=(ti == 0), stop=(ti == 3))
            mm = attn.tile([128, 128], BF16, tag=f"mm{h}{j}", name="mm")
            nc.vector.tensor_copy(mm[:], mm_ps[:])
            mv = attn.tile([1, 128], BF16, tag=f"mv{h}{j}", name="mv")
            nc.vector.tensor_copy(mv[:], mv_ps[:])
            mm_raw[(h, j)] = mm
            mv_raw[(h, j)] = mv

    def emit_trp(j, heads=None):
        for h in (range(H) if heads is None else heads):
            trp_f32 = ps_trp.tile([128, 512], F32, tag="trp", name="trp_ps")
            trp_bf = trp_f32.bitcast(BF16)
            for ti in range(4):
                t = 4 * j + ti
                sl = slice(t * DIM + h * 128, t * DIM + h * 128 + 128)
                nc.tensor.transpose(trp_bf[:, ti * 128:(ti + 1) * 128],
                                    Fq[:, sl], identbf[:])
            c0 = 4 * j * 8 + h
            wq_ps = trp_bf[0:4, 512:640]
            nc.tensor.transpose(wq_ps, qrb[:, c0:c0 + 25:8], identbf[:])
            fqTs = attn.tile([128, 512], BF16, tag=f"fqT{h}{j}", name="fqTs")
            nc.scalar.copy(fqTs[:, 0:256], trp_bf[:, 0:256])
            nc.vector.tensor_copy(fqTs[:, 256:512], trp_bf[:, 256:512])
            wq4 = trsc.tile([4, 128], BF16, tag="wq4", name="wq4")
            nc.scalar.copy(wq4[:], wq_ps)
            wqr = attn.tile([1, 512], BF16, tag=f"wqr{h}{j}", name="wqr")
            nc.scalar.dma_start(wqr[:], wq4[:])
            fqT_tiles[(h, j)] = fqTs
            wqr_tiles[(h, j)] = wqr

    with tc.tile_pool(name="ph1", bufs=1) as ph1, \
         tc.tile_pool(name="xpool", bufs=3) as xpool, \
         tc.tile_pool(name="spool", bufs=3) as spool, \
         tc.tile_pool(name="bnpool", bufs=2) as bnpool, \
         tc.tile_pool(name="ps_p1", bufs=3, space="PSUM") as ps_p1:
        Wp = ph1.tile([128, 8 * DIM], BF16)

        def wp_chunk(ss, q):
            q.dma_start(Wp[:, ss * DIM:(ss + 1) * DIM],
                        Wp_d[:, ss * DIM:(ss + 1) * DIM])

        x0 = {}
        xT0s = [xpool.tile([128, DIM], BF16, tag="xT", name=f"xT0_{i}")
                for i in range(3)]
        xn0s = [xpool.tile([128, DIM], BF16, tag="xn", name=f"xn0_{i}")
                for i in range(3)]
        # sync: Wp0, xTq, Wp3, Wp6, xTk ; scalar: Wp1, Wp4, xn*, Wp7 ;
        # gpsimd: Wp2, xTv, Wp5  (first group's chunks arrive in MM order)
        wp_chunk(0, nc.sync)
        wp_chunk(1, nc.scalar)
        wp_chunk(2, nc.gpsimd)
        nc.sync.dma_start(xT0s[0][:, 0:512], xTs[0][:, 0:512])
        nc.sync.dma_start(xT0s[0][:, 512:1024], xTs[0][:, 512:1024])
        wp_chunk(3, nc.sync)
        wp_chunk(4, nc.scalar)
        wp_chunk(5, nc.gpsimd)
        nc.gpsimd.dma_start(xT0s[2][:, 0:512], xTs[2][:, 0:512])
        nc.gpsimd.dma_start(xT0s[2][:, 512:1024], xTs[2][:, 512:1024])
        wp_chunk(6, nc.sync)
        wp_chunk(7, nc.scalar)
        nc.sync.dma_start(xT0s[1][:, 0:512], xTs[1][:, 0:512])
        nc.sync.dma_start(xT0s[1][:, 512:1024], xTs[1][:, 512:1024])
        for i in range(3):
            nc.scalar.dma_start(xn0s[i][:], xns[i][0:128, :])
            x0[i] = (xT0s[i], xn0s[i])
        WoT = late.tile([128, 8 * DIM], BF16)
        for t in range(NT):
            st = spool.tile([128, 6], F32, tag="st")
            bn6 = spool.tile([128, 36], F32, tag="bn6")
            sg = spool.tile([128, 3], F32, tag="sg")
            rsig = spool.tile([128, 3], F32, tag="rsig")
            bnq = bnpool.tile([128, 48], F32, tag="bnq")
            bnk = bnpool.tile([128, 48], F32, tag="bnk")
            for i in range(3):
                # LN stats for this tensor only -> per-tensor rsig, so each
                # tensor's evictions gate only on its own xn load
                if t == 0:
                    xn = x0[i][1]
                else:
                    xn = xpool.tile([128, DIM], BF16, tag="xn")
                    nc.scalar.dma_start(xn[:],
                                        xns[i][t * 128:(t + 1) * 128, :])
                nc.vector.bn_stats(bn6[:, i * 12:i * 12 + 6], xn[:, 0:512])
                nc.vector.bn_stats(bn6[:, i * 12 + 6:i * 12 + 12],
                                   xn[:, 512:1024])
                nc.vector.bn_aggr(st[:, 2 * i:2 * i + 2],
                                  bn6[:, i * 12:i * 12 + 12])
                nc.scalar.activation(sg[:, i:i + 1],
                                     st[:, 2 * i + 1:2 * i + 2],
                                     AF.Sqrt, bias=eps[:])
                nc.vector.reciprocal(rsig[:, i:i + 1], sg[:, i:i + 1])
                if t == 0:
                    xT_t = x0[i][0]
                else:
                    xT_t = xpool.tile([128, DIM], BF16, tag="xT")
                    qa_, qb_ = ((nc.sync, nc.sync), (nc.sync, nc.sync),
                                (nc.gpsimd, nc.gpsimd))[i]
                    qa_.dma_start(xT_t[:, 0:512],
                                  xTs[i][:, t * DIM:t * DIM + 512])
                    qb_.dma_start(xT_t[:, 512:1024],
                                  xTs[i][:, t * DIM + 512:(t + 1) * DIM])
                for half in range(2):
                    o = half * 512
                    acc = ps_p1.tile([128, 512], F32, tag="p1")
                    for s in range(8):
                        nc.tensor.matmul(
                            acc[:], xT_t[:, s * 128:(s + 1) * 128],
                            Wp[:, s * DIM + o: s * DIM + o + 512],
                            start=(s == 0),
                            stop=(s == 7 and not has_bias))
                    if has_bias:
                        nc.tensor.matmul(acc[:], onebf_row[:],
                                         vrow[:, o:o + 512],
                                         start=False, stop=True)
                    dst = Fs[i][:, t * DIM + o: t * DIM + o + 512]
                    nc.scalar.mul(dst, acc[:], rsig[:, i:i + 1])
                # F stats as soon as this tensor's tiles land (tail latency
                # of the last tile gates feat_corr / the M stage)
                if i == 0:
                    for h in range(H):
                        nc.vector.bn_stats(
                            bnq[:, h * 6:h * 6 + 6],
                            Fq[:, t * DIM + h * 128:t * DIM + h * 128 + 128])
                    for h in range(H):
                        c = t * 8 + h
                        nc.vector.bn_aggr(qa[:, 2 * c:2 * c + 2],
                                          bnq[:, h * 6:h * 6 + 6])
                    nc.vector.tensor_scalar_mul(qmean[:, t * 8:t * 8 + 8],
                                                qa[:, 16 * t:16 * t + 16:2],
                                                -1.0)
                    qeng = nc.vector if t == NT - 1 else nc.gpsimd
                    for h in range(H):
                        c = t * 8 + h
                        fsl = slice(t * DIM + h * 128,
                                    t * DIM + h * 128 + 128)
                        qc = qcpool.tile([128, 128], BF16, tag="qc",
                                         name="qc")
                        qeng.tensor_scalar(qc[:], Fq[:, fsl],
                                           qmean[:, c:c + 1], None,
                                           ALU.add)
                        qc_tiles[(t, h)] = qc
                elif i == 1:
                    for h in range(H):
                        nc.vector.bn_stats(
                            bnk[:, h * 6:h * 6 + 6],
                            Fk[:, t * DIM + h * 128:t * DIM + h * 128 + 128])
                    for h in range(H):
                        c = t * 8 + h
                        nc.vector.bn_aggr(ka[:, 2 * c:2 * c + 2],
                                          bnk[:, h * 6:h * 6 + 6])
            nc.scalar.dma_start(WoT[:, t * DIM:(t + 1) * DIM],
                                WoT_d[:, t * DIM:(t + 1) * DIM])

            if t in (3, 7):
                jh = t // 4
                sl = slice(32 * jh, 32 * jh + 32)
                derived(qa, qninv, qrb, jh)
                derived(ka, kninv, kr, jh, n_out=kn)
                nc.vector.tensor_tensor(krkn[:, sl], kr[:, sl], kn[:, sl],
                                        op=ALU.mult)
                tts = range(4) if jh == 0 else range(7, 8)
                for tt in tts:
                    for h in range(H):
                        c = tt * 8 + h
                        fsl = slice(tt * DIM + h * 128,
                                    tt * DIM + h * 128 + 128)
                        nc.vector.tensor_scalar(
                            Fv[:, fsl], Fv[:, fsl],
                            kninv[:, c:c + 1], None, ALU.mult)
            # task-0 attention-core work rides inside phase 1: its PE bursts
            # interleave with the dense projection stream and its evictions
            # land on engine slack
            if t == 5:
                emit_gk(0)
                emit_fqscale(0)
                emit_mmv(0)
            elif t == 6:
                emit_trp(0)

    # ---- remaining constants (first used after phase 1) ----
    ident8 = consts.tile([8, 8], F32)
    nc.sync.dma_start(ident8[:], ident_d[0:8, 0:8])
    ones = consts.tile([128, 8], F32)
    nc.sync.dma_start(ones[:], ones_d[:, 0:8])
    ones8 = consts.tile([1, 8], F32)
    nc.sync.dma_start(ones8[:], ones_d[0:1, 0:8])
    mask_nd = consts.tile([128, H * 128], BF16)
    nc.scalar.dma_start(mask_nd[:], mask_d[:])
    wp1T = consts.tile([128, 256], F32)
    nc.scalar.dma_start(wp1T[:], wp1T_d[:])
    wp2T = consts.tile([128, 3], F32)
    nc.scalar.dma_start(wp2T[:], wp2T_d[:])
    b1row = consts.tile([1, 128], F32)
    nc.scalar.dma_start(b1row[:], b1_d[:])
    gbc = consts.tile([8, 128], F32)
    nc.scalar.dma_start(gbc[:], gbc_d[:])
    bbc = consts.tile([8, 128], F32)
    nc.scalar.dma_start(bbc[:], bbc_d[:])
    b2bc = consts.tile([8, 3], F32)
    nc.scalar.dma_start(b2bc[:], b2bc_d[:])

    # ======== post-phase-1: task-1 region + collectives ========
    ps_small = pre.enter_context(tc.tile_pool(name="ps_small", bufs=1,
                                              space="PSUM"))
    emit_gk(1)
    gk_sb = trsc.tile([128, 16], BF16, tag="gksb", name="gk_sb")
    nc.scalar.copy(gk_sb[:], gk_ps[:])
    nc.sync.dma_start(ar_in_g[:], gk_sb[:])
    if n_cores > 1:
        nc.gpsimd.collective_compute(
            "AllReduce", ALU.add,
            replica_groups=[list(range(n_cores))],
            ins=[ar_in_g.opt()], outs=[ar_out_g.opt()])
    else:
        nc.sync.dma_start(ar_out_g[:], ar_in_g[:])
    arg = late.tile([128, 16], BF16)
    nc.scalar.dma_start(arg[:], ar_out_g[:])

    # feat_corr partials per head, shipped as they complete
    for h in range(H):
        fc_ps = ps_fc.tile([128, 128], F32, tag="fc128", name="fc_ps")
        for t in range(NT):
            nc.tensor.matmul(fc_ps[:], qc_tiles[(t, h)][:],
                             qc_tiles[(t, h)][:],
                             start=(t == 0), stop=(t == NT - 1))
        fc_sb = trsc.tile([128, 128], BF16, tag="fcsb", name="fc_sb")
        nc.scalar.copy(fc_sb[:], fc_ps[:])
        nc.sync.dma_start(ar_in_fc[:, h * 128:(h + 1) * 128], fc_sb[:])
    if n_cores > 1:
        nc.gpsimd.collective_compute(
            "AllReduce", ALU.add,
            replica_groups=[list(range(n_cores))],
            ins=[ar_in_fc.opt()], outs=[ar_out_fc.opt()])
    else:
        nc.sync.dma_start(ar_out_fc[:], ar_in_fc[:])
    ar = late.tile([128, H * 128], BF16)
    nc.sync.dma_start(ar[:], ar_out_fc[:])

    emit_fqscale(1)
    emit_mmv(1)

    # ======== Phase 3a: weight predictor (needs only gk slice) ========
    featsq = stat1.tile([128, 8], F32)
    nc.vector.tensor_scalar_mul(featsq[:], arg[:, 0:8], 1.0 / TOK_ALL)
    featsk = stat1.tile([128, 8], F32)
    nc.vector.tensor_scalar_mul(featsk[:], arg[:, 8:16], 1.0 / TOK_ALL)
    h1_ps = ps_small.tile([8, 128], F32, tag="sm", name="h1_ps")
    nc.tensor.matmul(h1_ps[:], featsq[:], wp1T[:, 0:128], start=True,
                     stop=False)
    nc.tensor.matmul(h1_ps[:], featsk[:], wp1T[:, 128:256], start=False,
                     stop=False)
    nc.tensor.matmul(h1_ps[:], ones8[:], b1row[:], start=False, stop=True)
    h1 = stat1.tile([8, 128], F32)
    nc.scalar.copy(h1[:], h1_ps[:])
    w_mu = stat1.tile([8, 4], F32)
    sq8 = stat1.tile([8, 128], F32)
    nc.vector.reduce_sum(w_mu[:, 0:1], h1[:], axis=AX.X)
    nc.vector.tensor_scalar_mul(w_mu[:, 0:1], w_mu[:, 0:1], 1.0 / D)
    nc.scalar.activation(sq8[:], h1[:], AF.Square, accum_out=w_mu[:, 1:2])
    nc.vector.tensor_scalar_mul(w_mu[:, 1:2], w_mu[:, 1:2], 1.0 / D)
    nc.vector.tensor_tensor(w_mu[:, 2:3], w_mu[:, 0:1], w_mu[:, 0:1],
                            op=ALU.mult)
    nc.vector.tensor_tensor(w_mu[:, 2:3], w_mu[:, 1:2], w_mu[:, 2:3],
                            op=ALU.subtract)
    nc.scalar.activation(w_mu[:, 3:4], w_mu[:, 2:3], AF.Sqrt, bias=eps[0:8, :])
    nc.vector.reciprocal(w_mu[:, 3:4], w_mu[:, 3:4])
    h1n = stat1.tile([8, 128], F32)
    nc.vector.tensor_scalar(h1n[:], h1[:], w_mu[:, 0:1], w_mu[:, 3:4],
                            ALU.subtract, ALU.mult)
    nc.vector.tensor_tensor(h1n[:], h1n[:], gbc[:], op=ALU.mult)
    nc.vector.tensor_tensor(h1n[:], h1n[:], bbc[:], op=ALU.add)
    nc.vector.tensor_scalar_max(h1n[:], h1n[:], 0.0)
    h1T_ps = ps_small.tile([128, 8], F32, tag="sm", name="h1T_ps")
    nc.tensor.transpose(h1T_ps[:], h1n[:], ident8[:])
    h1T = stat1.tile([128, 8], F32)
    nc.scalar.copy(h1T[:], h1T_ps[:])
    lg_ps = ps_small.tile([8, 3], F32, tag="sm", name="lg_ps")
    nc.tensor.matmul(lg_ps[:], h1T[:], wp2T[:], start=True, stop=True)
    lg = stat1.tile([8, 8], F32)
    nc.scalar.copy(lg[:, 0:3], lg_ps[:])
    nc.vector.tensor_tensor(lg[:, 0:3], lg[:, 0:3], b2bc[:], op=ALU.add)
    # logits are O(1): skip the (mathematically redundant) max-subtraction
    nc.scalar.activation(lg[:, 0:3], lg[:, 0:3], AF.Exp)
    nc.vector.reduce_sum(lg[:, 4:5], lg[:, 0:3], axis=AX.X)
    nc.vector.reciprocal(lg[:, 4:5], lg[:, 4:5])
    nc.vector.tensor_scalar(lg[:, 0:3], lg[:, 0:3], lg[:, 4:5], None,
                            ALU.mult)



    # ======== Phase 3b: decorr scale (needs feat_corr block) ========
    ssq = stat1.tile([128, 8], F32)
    msk = late.tile([128, H * 128], BF16)
    nc.vector.tensor_tensor(msk[:], ar[:], mask_nd[:], op=ALU.mult)
    sqf = late.tile([128, H * 128], F32)
    nc.scalar.activation(sqf[:], msk[:], AF.Square)
    nc.vector.reduce_sum(ssq[:],
                         sqf[:].rearrange("p (h d) -> p h d", h=8),
                         axis=AX.X)
    ss_ps = ps_small.tile([8, 8], F32, tag="sm", name="ss_ps")
    nc.tensor.matmul(ss_ps[:], ssq[:], ones[:, 0:8], start=True, stop=True)
    dsc = stat1.tile([8, 8], F32)
    nc.scalar.activation(dsc[:, 0:1], ss_ps[0:8, 0:1], AF.Sqrt)
    nc.scalar.activation(dsc[:, 1:2], dsc[:, 0:1], AF.Exp,
                         scale=-5.0 / (D * D * TOK_ALL))

    # alpha = w0 + w1*dsc ; wsc = w2/alpha ; flat row [alpha(8) | wsc(8)]
    aw = stat1.tile([8, 4], F32)
    nc.vector.tensor_tensor(aw[:, 0:1], lg[:, 1:2], dsc[:, 1:2], op=ALU.mult)
    nc.vector.tensor_tensor(aw[:, 0:1], aw[:, 0:1], lg[:, 0:1], op=ALU.add)
    nc.vector.reciprocal(aw[:, 2:3], aw[:, 0:1])
    nc.vector.tensor_tensor(aw[:, 1:2], lg[:, 2:3], aw[:, 2:3], op=ALU.mult)
    awT_ps = ps_small.tile([1, 8], F32, tag="sm", name="awT_ps")
    nc.tensor.transpose(awT_ps[:], aw[:, 0:1], ident8[:])
    awTa = stat1.tile([1, 8], F32)
    nc.scalar.copy(awTa[:], awT_ps[:])
    awT_ps2 = ps_small.tile([1, 8], F32, tag="sm", name="awT_ps2")
    nc.tensor.transpose(awT_ps2[:], aw[:, 1:2], ident8[:])
    awTb = stat1.tile([1, 8], F32)
    nc.scalar.copy(awTb[:], awT_ps2[:])
    abc = stat1.tile([128, 8], F32)
    nc.gpsimd.partition_broadcast(abc[:], awTa[:])
    wscbc = stat1.tile([128, 8], F32)
    nc.gpsimd.partition_broadcast(wscbc[:], awTb[:])
    pre.close()

    # ======== Phase 4b + 5: scaled attention + output projection ========
    with tc.tile_pool(name="ph4", bufs=2) as ph4, \
         tc.tile_pool(name="o1pool", bufs=10) as o1pool, \
         tc.tile_pool(name="ps_o1", bufs=3, space="PSUM") as ps_o1, \
         tc.tile_pool(name="ps_p5", bufs=2, space="PSUM") as ps_p5:
        o1_tiles = {}
        for j in range(NTASK):
            for h in range(H):
                # mv row scaled by ww/alpha (tiny); alpha applied at eviction
                mvw = ph4.tile([1, 128], BF16, tag="mvw", name="mvw")
                nc.vector.tensor_scalar(mvw[:], mv_raw[(h, j)][:],
                                        wscbc[0:1, h:h + 1], None,
                                        ALU.mult)
                o1_ps = ps_o1.tile([128, 512], F32, tag="o1", name="o1_ps")
                nc.tensor.matmul(o1_ps[:], mm_raw[(h, j)][:],
                                 fqT_tiles[(h, j)][:], start=True, stop=False)
                nc.tensor.matmul(o1_ps[:], mvw[:], wqr_tiles[(h, j)][:],
                                 start=False, stop=True)
                o1 = o1pool.tile([128, 512], BF16, tag="o1sb", name="o1_sb")
                if h % 2 == 0:
                    nc.vector.tensor_scalar(o1[:], o1_ps[:], abc[:, h:h + 1],
                                            None, ALU.mult)
                else:
                    nc.scalar.mul(o1[:], o1_ps[:], abc[:, h:h + 1])
                o1_tiles[(h, j)] = o1

            # ---- output projection for this task ----
            for t in range(4 * j, 4 * j + 4):
                ti = t % 4
                for half in range(2):
                    o = half * 512
                    op_ps = ps_p5.tile([128, 512], F32, tag="p5",
                                       name="op_ps")
                    for h in range(H):
                        nc.tensor.matmul(
                            op_ps[:],
                            o1_tiles[(h, j)][:, ti * 128:(ti + 1) * 128],
                            WoT[:, h * DIM + o: h * DIM + o + 512],
                            start=(h == 0),
                            stop=(h == H - 1 and not has_bias))
                    if has_bias:
                        nc.tensor.matmul(op_ps[:], onebf_row[:],
                                         bout[:, o:o + 512],
                                         start=False, stop=True)
                    ysb = ph4.tile([128, 512], F32, tag="ysb", name="ysb")
                    if j == 1 and (t + half) % 2 == 1:
                        nc.vector.tensor_copy(ysb[:], op_ps[:])
                    else:
                        nc.scalar.copy(ysb[:], op_ps[:])
                    qy = nc.sync if (t + half) % 2 == 0 else nc.scalar
                    qy.dma_start(y[t * 128:(t + 1) * 128, o:o + 512],
                                 ysb[:])


_BUILT = {}


def _build(n_cores=N_CORES, has_bias=False):
    key = (n_cores, has_bias)
    if key in _BUILT:
        return _BUILT[key]
    nc = bacc.Bacc("TRN2", target_bir_lowering=False, debug=False,
                   num_devices=n_cores)
    in_specs = [
        ("xn_q", [T, DIM], BF16), ("xn_k", [T, DIM], BF16),
        ("xn_v", [T, DIM], BF16),
        ("xT_q", [128, NT * DIM], BF16), ("xT_k", [128, NT * DIM], BF16),
        ("xT_v", [128, NT * DIM], BF16),
        ("Wp", [128, 8 * DIM], BF16), ("WoT", [128, 8 * DIM], BF16),
        ("vrow", [1, DIM], BF16), ("bout", [1, DIM], BF16),
        ("ones", [128, 128], F32), ("onesbf", [128, 8], BF16),
        ("identbf", [128, 128], BF16), ("ident", [128, 128], F32),
        ("mask", [128, 1024], BF16),
        ("wp1T", [128, 256], F32), ("wp2T", [128, 3], F32),
        ("b1row", [1, 128], F32),
        ("gbc", [8, 128], F32), ("bbc", [8, 128], F32), ("b2bc", [8, 3], F32),
    ]
    in_aps = [nc.dram_tensor(n, s, dt, kind="ExternalInput").ap()
              for n, s, dt in in_specs]
    y_ap = nc.dram_tensor("y", [T, DIM], F32, kind="ExternalOutput").ap()
    with tile.TileContext(nc) as tc:
        attn_kernel(tc, [y_ap], in_aps, n_cores=n_cores, has_bias=has_bias)
    nc.compile()
    _BUILT[key] = nc
    return nc


def _bf(x):
    import ml_dtypes
    return np.asarray(x, dtype=ml_dtypes.bfloat16)


def kernel(q, k, v, ln_g, ln_b, w_in, wp_w1, wp_b1, wp_ln_g, wp_ln_b,
           wp_w2, wp_b2, w_out, b_out):
    q = np.asarray(q, dtype=np.float32)
    k = np.asarray(k, dtype=np.float32)
    v = np.asarray(v, dtype=np.float32)
    ln_g = np.asarray(ln_g, np.float32); ln_b = np.asarray(ln_b, np.float32)
    w_in = np.asarray(w_in, np.float32); w_out = np.asarray(w_out, np.float32)
    b_out = np.asarray(b_out, np.float32)
    wp_w1 = np.asarray(wp_w1, np.float32); wp_b1 = np.asarray(wp_b1, np.float32)
    wp_ln_g = np.asarray(wp_ln_g, np.float32)
    wp_ln_b = np.asarray(wp_ln_b, np.float32)
    wp_w2 = np.asarray(wp_w2, np.float32); wp_b2 = np.asarray(wp_b2, np.float32)

    # host weight prep: fold LN gain into W, then column-center so x @ Wp
    # carries the -mu*sum(g*W) correction implicitly
    W = w_in.T                                     # [DIM, HD]
    Wp = (ln_g[:, None] * W)
    Wp = Wp - Wp.mean(axis=0, keepdims=True)
    vrow = (ln_b @ W)[None, :]
    has_bias = bool(np.any(ln_b != 0.0) or np.any(b_out != 0.0))
    Wp_t = np.ascontiguousarray(
        Wp.reshape(8, 128, 2, 512).transpose(1, 0, 2, 3)).reshape(128, -1)
    WoT = np.ascontiguousarray(
        w_out.T.reshape(8, 128, DIM).transpose(1, 0, 2)).reshape(128, -1)
    shared = {
        "Wp": _bf(Wp_t), "WoT": _bf(WoT), "vrow": _bf(vrow),
        "bout": _bf(b_out[None, :]),
        "ones": np.ones((128, 128), np.float32),
        "onesbf": _bf(np.ones((128, 8), np.float32)),
        "identbf": _bf(np.eye(128, dtype=np.float32)),
        "ident": np.eye(128, dtype=np.float32),
        "mask": _bf(np.tile((1.0 - np.eye(128)).astype(np.float32), (1, 8))),
        "wp1T": np.ascontiguousarray(wp_w1.T.reshape(2, 128, 128)
                                     .transpose(1, 0, 2)).reshape(128, 256),
        "wp2T": np.ascontiguousarray(wp_w2.T),
        "b1row": wp_b1[None, :],
        "gbc": np.tile(wp_ln_g[None, :], (8, 1)),
        "bbc": np.tile(wp_ln_b[None, :], (8, 1)),
        "b2bc": np.tile(wp_b2[None, :], (8, 1)),
    }
    for kk in ("ones", "ident", "wp1T", "wp2T", "b1row", "gbc", "bbc",
               "b2bc"):
        shared[kk] = np.ascontiguousarray(shared[kk], np.float32)

    qf = q.reshape(QB * N, DIM)
    kf = k.reshape(QB * N, DIM)
    vf = v.reshape(QB * N, DIM)
    in_maps = []
    for c in range(N_CORES):
        sl = slice(c * T, (c + 1) * T)
        m = dict(shared)
        for nm, arr in (("q", qf[sl]), ("k", kf[sl]), ("v", vf[sl])):
            m[f"xn_{nm}"] = _bf(np.ascontiguousarray(arr))
            m[f"xT_{nm}"] = _bf(np.ascontiguousarray(
                arr.reshape(NT, 128, 8, 128).transpose(3, 0, 2, 1)
            ).reshape(128, NT * DIM))
        in_maps.append(m)

    nc = _build(has_bias=has_bias)
    res = bass_utils.run_bass_kernel_spmd(nc, in_maps,
                                          core_ids=list(range(N_CORES)))
    global LAST_RESULTS
    LAST_RESULTS = res
    out = np.concatenate([np.asarray(r["y"], np.float32)
                          for r in res.results], axis=0)
    return out.reshape(QB, N, DIM)


LAST_RESULTS = None


# revision 5
# speedup vs baseline: 1.0223x; 1.0034x over previous
"""Trainium2 Bass kernel for nn_Attention_9096740733536 (sparse_attention).

Sharding: data-parallel over the QB (task) dim across 8 cores (2 tasks/core),
one mid-kernel AllReduce of [feat_corr partials | q_global | k_global] sums
(bf16 payload). The attention math is algebraically collapsed: mixed scores
are linear (no softmax), so
  out[h,q] = alpha_h*(Fq/qn) @ ((Fk/kn)^T @ Fv) + ww_h*q_ratio (x) (k_ratio^T Fv)
with 128x128 inner matrices instead of 512x512 score matrices. LayerNorm's
mean-correction is folded into the projection weights on the host (column
centering); the 1/sigma scale is applied at PSUM eviction. All PE operands
are bf16 (fp32 PSUM accumulation). Per-token/head stats come from batched
bn_stats (DVE); centering and the Fv/kn scale run on GPSIMD to keep ACT/DVE
off the critical path. Everything except the alpha/ww scaling runs before
the AllReduce; alpha is folded into the o1 eviction and ww/alpha into the
mv rows, so the post-collective tail is just o1 + output projection.
"""
import numpy as np
from contextlib import ExitStack

import concourse.bass as bass
import concourse.tile as tile
from concourse import bacc, mybir
from concourse import bass_utils
from concourse._compat import with_exitstack

F32 = mybir.dt.float32
BF16 = mybir.dt.bfloat16
AF = mybir.ActivationFunctionType
ALU = mybir.AluOpType
AX = mybir.AxisListType

H, D, DIM = 8, 128, 1024
QB, N = 16, 512
N_CORES = 8
T = QB * N // N_CORES          # 1024 tokens per core
NT = T // 128                  # 8 token tiles per core
NTASK = T // N                 # 2 tasks per core
LN_EPS = 1e-5
TOK_ALL = float(QB * N)


@with_exitstack
def attn_kernel(ctx: ExitStack, tc: tile.TileContext, outs, ins,
                n_cores=N_CORES, has_bias=False):
    nc = tc.nc
    y = outs[0]
    (xn_q, xn_k, xn_v, xT_q, xT_k, xT_v, Wp_d, WoT_d, vrow_d, bout_d,
     ones_d, onesbf_d, identbf_d, ident_d, mask_d, wp1T_d, wp2T_d, b1_d,
     gbc_d, bbc_d, b2bc_d) = ins

    consts = ctx.enter_context(tc.tile_pool(name="consts", bufs=1))
    fpool = ctx.enter_context(tc.tile_pool(name="fpool", bufs=1))
    stat1 = ctx.enter_context(tc.tile_pool(name="stat1", bufs=1))
    dram = ctx.enter_context(tc.tile_pool(name="dram", bufs=1, space="DRAM"))
    attn = ctx.enter_context(tc.tile_pool(name="attn", bufs=1))
    late = ctx.enter_context(tc.tile_pool(name="late", bufs=1))
    qcpool = ctx.enter_context(tc.tile_pool(name="qcpool", bufs=64))

    # PSUM banks: phase1 = p1(3)+trp(2)+fc(2)+gk(1); post-p1 the p1 pool
    # frees and ps_small(1) enters; phase 4b/5 run on o1(3)+p5(2).
    pre = ExitStack()
    ps_trp = pre.enter_context(tc.tile_pool(name="ps_trp", bufs=2,
                                            space="PSUM"))
    ps_fc = pre.enter_context(tc.tile_pool(name="ps_fc", bufs=2,
                                           space="PSUM"))
    ps_gk = pre.enter_context(tc.tile_pool(name="ps_gk", bufs=1,
                                           space="PSUM"))
    trsc = pre.enter_context(tc.tile_pool(name="trsc", bufs=2))

    # ---- constants needed inside phase 1 ----
    eps = consts.tile([128, 1], F32)
    nc.vector.memset(eps[:], LN_EPS)
    if has_bias:
        vrow = consts.tile([1, DIM], BF16)
        nc.sync.dma_start(vrow[:], vrow_d[:])
        bout = consts.tile([1, DIM], BF16)
        nc.sync.dma_start(bout[:], bout_d[:])
        onebf_row = consts.tile([1, 128], BF16)
        nc.vector.memset(onebf_row[:], 1.0)

    # ---- persistent F tensors: [128 tok, t*1024 + h*128 + d], bf16 ----
    Fq = fpool.tile([128, NT * DIM], BF16)
    Fk = fpool.tile([128, NT * DIM], BF16)
    Fv = fpool.tile([128, NT * DIM], BF16)

    xns = [xn_q, xn_k, xn_v]
    xTs = [xT_q, xT_k, xT_v]
    Fs = [Fq, Fk, Fv]

    # ---- per-(tile,head) stats: qa/ka hold (mean, var) pairs, col 2c/2c+1
    # for c = t*8+h; dense derived tiles are indexed by c ----
    qa = stat1.tile([128, 128], F32)
    ka = stat1.tile([128, 128], F32)
    qmean = stat1.tile([128, 64], F32)      # NEGATED mean (ACT/Pool bias)
    qninv = stat1.tile([128, 64], F32)
    kninv = stat1.tile([128, 64], F32)
    kn = stat1.tile([128, 64], F32)
    qrb = stat1.tile([128, 64], BF16)       # q_ratio (PE transpose input)
    krkn = stat1.tile([128, 64], BF16)      # k_ratio*kn (PE lhsT)
    kr = stat1.tile([128, 64], F32)
    rscr = stat1.tile([128, 128], F32)

    def derived(a, ninv, ratio, jh, n_out=None, negmean=None):
        # a: (m,v) pairs; group c in [32jh, 32jh+32)
        m = a[:, 64 * jh: 64 * jh + 64: 2]
        v = a[:, 64 * jh + 1: 64 * jh + 64: 2]
        sl = slice(32 * jh, 32 * jh + 32)
        t1 = rscr[:, 0:32]
        t2 = rscr[:, 32:64]
        t3 = rscr[:, 64:96]
        # qn = sqrt(D*(m^2 + v)) ; ninv = 1/qn
        nc.vector.tensor_tensor(t1, m, m, op=ALU.mult)
        nc.vector.tensor_tensor(t1, t1, v, op=ALU.add)
        if n_out is not None:
            nc.scalar.activation(n_out[:, sl], t1, AF.Sqrt, scale=float(D))
            nc.vector.reciprocal(ninv[:, sl], n_out[:, sl])
        else:
            nc.scalar.activation(ninv[:, sl], t1, AF.Sqrt, scale=float(D))
            nc.vector.reciprocal(ninv[:, sl], ninv[:, sl])
        # unbiased var vu = v*D/(D-1); ratio = 2*min(vu,1)/(vu+1)
        nc.vector.tensor_scalar_mul(t2, v, float(D) / (D - 1))
        nc.vector.tensor_scalar(t1, t2, 1.0, 2.0, ALU.min, ALU.mult)
        nc.vector.tensor_scalar_add(t3, t2, 1.0)
        nc.vector.reciprocal(t3, t3)
        nc.vector.tensor_tensor(ratio[:, sl], t1, t3, op=ALU.mult)
        if negmean is not None:
            nc.vector.tensor_scalar_mul(negmean[:, sl], m, -1.0)

    identbf = consts.tile([128, 128], BF16)
    nc.scalar.dma_start(identbf[:], identbf_d[:])
    onesbf = consts.tile([128, 8], BF16)
    nc.scalar.dma_start(onesbf[:], onesbf_d[:])

    # ======== Phase 1 (+ per-tile stats emission) ========
    qc_tiles = {}
    ar_in_g = dram.tile([128, 16], BF16)
    ar_out_g = dram.tile([128, 16], BF16)
    ar_in_fc = dram.tile([128, H * 128], BF16)
    ar_out_fc = dram.tile([128, H * 128], BF16)
    gk_ps = ps_gk.tile([128, 16], F32, tag="gk")
    arg = late.tile([128, 16], BF16)
    ar = late.tile([128, H * 128], BF16)
    mm_raw = {}
    mv_raw = {}
    fqT_tiles = {}
    wqr_tiles = {}

    def emit_gk(j):
        # one accumulation group spans both tasks (opened at t=0's chunk,
        # closed by the post-phase-1 chunk)
        for t in range(4 * j, 4 * j + 4):
            for h in range(H):
                sl = slice(t * DIM + h * 128, t * DIM + h * 128 + 128)
                first = (j == 0 and t == 0 and h == 0)
                last = (j == 1 and t == NT - 1 and h == H - 1)
                nc.tensor.matmul(gk_ps[:, h:h + 1],
                                 Fq[:, sl], onesbf[:, 0:1],
                                 start=first, stop=last,
                                 skip_group_check=True)
                nc.tensor.matmul(gk_ps[:, 8 + h:9 + h],
                                 Fk[:, sl], onesbf[:, 0:1],
                                 start=False, stop=False,
                                 skip_group_check=True)

    def emit_fqscale(j):
        # in-place Fq <- Fq/qn (after gk + qc reads of raw Fq)
        for h in range(H):
            seng = nc.vector if h % 2 == 0 else nc.gpsimd
            for t in range(4 * j, 4 * j + 4):
                sl = slice(t * DIM + h * 128, t * DIM + h * 128 + 128)
                c = slice(t * 8 + h, t * 8 + h + 1)
                seng.tensor_scalar(Fq[:, sl], Fq[:, sl], qninv[:, c],
                                   None, ALU.mult)

    def emit_mmv(j, heads=None):
        for h in (range(H) if heads is None else heads):
            mm_ps = ps_fc.tile([128, 128], F32, tag="fc128", name="mm_ps")
            trp_f32 = ps_trp.tile([128, 512], F32, tag="trp", name="trp_ps")
            mv_ps = trp_f32[0:1, 320:448]
            for ti in range(4):
                t = 4 * j + ti
                sl = slice(t * DIM + h * 128, t * DIM + h * 128 + 128)
                nc.tensor.matmul(mm_ps[:], Fk[:, sl], Fv[:, sl],
                                 start=(ti == 0), stop=(ti == 3))
                nc.tensor.matmul(mv_ps[:],
                                 krkn[:, t * 8 + h:t * 8 + h + 1],
                                 Fv[:, sl], start=(ti == 0), stop=(ti == 3))
            mm = attn.tile([128, 128], BF16, tag=f"mm{h}{j}", name="mm")
            nc.vector.tensor_copy(mm[:], mm_ps[:])
            mv = attn.tile([1, 128], BF16, tag=f"mv{h}{j}", name="mv")
            nc.vector.tensor_copy(mv[:], mv_ps[:])
            mm_raw[(h, j)] = mm
            mv_raw[(h, j)] = mv

    def emit_trp(j, heads=None):
        for h in (range(H) if heads is None else heads):
            trp_f32 = ps_trp.tile([128, 512], F32, tag="trp", name="trp_ps")
            trp_bf = trp_f32.bitcast(BF16)
            for ti in range(4):
                t = 4 * j + ti
                sl = slice(t * DIM + h * 128, t * DIM + h * 128 + 128)
                nc.tensor.transpose(trp_bf[:, ti * 128:(ti + 1) * 128],
                                    Fq[:, sl], identbf[:])
            c0 = 4 * j * 8 + h
            wq_ps = trp_bf[0:4, 512:640]
            nc.tensor.transpose(wq_ps, qrb[:, c0:c0 + 25:8], identbf[:])
            fqTs = attn.tile([128, 512], BF16, tag=f"fqT{h}{j}", name="fqTs")
            nc.scalar.copy(fqTs[:, 0:256], trp_bf[:, 0:256])
            nc.vector.tensor_copy(fqTs[:, 256:512], trp_bf[:, 256:512])
            wq4 = trsc.tile([4, 128], BF16, tag="wq4", name="wq4")
            nc.scalar.copy(wq4[:], wq_ps)
            wqr = attn.tile([1, 512], BF16, tag=f"wqr{h}{j}", name="wqr")
            nc.scalar.dma_start(wqr[:], wq4[:])
            fqT_tiles[(h, j)] = fqTs
            wqr_tiles[(h, j)] = wqr

    with tc.tile_pool(name="ph1", bufs=1) as ph1, \
         tc.tile_pool(name="xpool", bufs=3) as xpool, \
         tc.tile_pool(name="spool", bufs=3) as spool, \
         tc.tile_pool(name="bnpool", bufs=2) as bnpool, \
         tc.tile_pool(name="ps_p1", bufs=3, space="PSUM") as ps_p1:
        Wp = ph1.tile([128, 8 * DIM], BF16)

        def wp_chunk(ss, q):
            q.dma_start(Wp[:, ss * DIM:(ss + 1) * DIM],
                        Wp_d[:, ss * DIM:(ss + 1) * DIM])

        x0 = {}
        xT0s = [xpool.tile([128, DIM], BF16, tag="xT", name=f"xT0_{i}")
                for i in range(3)]
        xn0s = [xpool.tile([128, DIM], BF16, tag="xn", name=f"xn0_{i}")
                for i in range(3)]
        # sync: Wp0, xTq, Wp3, Wp6, xTk ; scalar: Wp1, Wp4, xn*, Wp7 ;
        # gpsimd: Wp2, xTv, Wp5  (first group's chunks arrive in MM order)
        wp_chunk(0, nc.sync)
        wp_chunk(1, nc.scalar)
        wp_chunk(2, nc.gpsimd)
        nc.sync.dma_start(xT0s[0][:, 0:512], xTs[0][:, 0:512])
        nc.sync.dma_start(xT0s[0][:, 512:1024], xTs[0][:, 512:1024])
        wp_chunk(3, nc.sync)
        wp_chunk(4, nc.scalar)
        wp_chunk(5, nc.gpsimd)
        nc.gpsimd.dma_start(xT0s[2][:, 0:512], xTs[2][:, 0:512])
        nc.gpsimd.dma_start(xT0s[2][:, 512:1024], xTs[2][:, 512:1024])
        wp_chunk(6, nc.sync)
        wp_chunk(7, nc.scalar)
        nc.sync.dma_start(xT0s[1][:, 0:512], xTs[1][:, 0:512])
        nc.sync.dma_start(xT0s[1][:, 512:1024], xTs[1][:, 512:1024])
        for i in range(3):
            nc.scalar.dma_start(xn0s[i][:], xns[i][0:128, :])
            x0[i] = (xT0s[i], xn0s[i])
        WoT = late.tile([128, 8 * DIM], BF16)
        for t in range(NT):
            st = spool.tile([128, 6], F32, tag="st")
            bn6 = spool.tile([128, 36], F32, tag="bn6")
            sg = spool.tile([128, 3], F32, tag="sg")
            rsig = spool.tile([128, 3], F32, tag="rsig")
            bnq = bnpool.tile([128, 48], F32, tag="bnq")
            bnk = bnpool.tile([128, 48], F32, tag="bnk")
            for i in range(3):
                # LN stats for this tensor only -> per-tensor rsig, so each
                # tensor's evictions gate only on its own xn load
                if t == 0:
                    xn = x0[i][1]
                else:
                    xn = xpool.tile([128, DIM], BF16, tag="xn")
                    nc.scalar.dma_start(xn[:],
                                        xns[i][t * 128:(t + 1) * 128, :])
                nc.vector.bn_stats(bn6[:, i * 12:i * 12 + 6], xn[:, 0:512])
                nc.vector.bn_stats(bn6[:, i * 12 + 6:i * 12 + 12],
                                   xn[:, 512:1024])
                nc.vector.bn_aggr(st[:, 2 * i:2 * i + 2],
                                  bn6[:, i * 12:i * 12 + 12])
                nc.scalar.activation(sg[:, i:i + 1],
                                     st[:, 2 * i + 1:2 * i + 2],
                                     AF.Sqrt, bias=eps[:])
                nc.vector.reciprocal(rsig[:, i:i + 1], sg[:, i:i + 1])
                if t == 0:
                    xT_t = x0[i][0]
                else:
                    xT_t = xpool.tile([128, DIM], BF16, tag="xT")
                    qa_, qb_ = ((nc.sync, nc.sync), (nc.sync, nc.sync),
                                (nc.gpsimd, nc.gpsimd))[i]
                    qa_.dma_start(xT_t[:, 0:512],
                                  xTs[i][:, t * DIM:t * DIM + 512])
                    qb_.dma_start(xT_t[:, 512:1024],
                                  xTs[i][:, t * DIM + 512:(t + 1) * DIM])
                for half in range(2):
                    o = half * 512
                    acc = ps_p1.tile([128, 512], F32, tag="p1")
                    for s in range(8):
                        nc.tensor.matmul(
                            acc[:], xT_t[:, s * 128:(s + 1) * 128],
                            Wp[:, s * DIM + o: s * DIM + o + 512],
                            start=(s == 0),
                            stop=(s == 7 and not has_bias))
                    if has_bias:
                        nc.tensor.matmul(acc[:], onebf_row[:],
                                         vrow[:, o:o + 512],
                                         start=False, stop=True)
                    dst = Fs[i][:, t * DIM + o: t * DIM + o + 512]
                    nc.scalar.mul(dst, acc[:], rsig[:, i:i + 1])
                # F stats as soon as this tensor's tiles land (tail latency
                # of the last tile gates feat_corr / the M stage)
                if i == 0:
                    if t == NT - 1:
                        # last tile: per-head stats->qc chains so feat_corr's
                        # per-head gates open incrementally
                        for h in range(H):
                            c = t * 8 + h
                            nc.vector.bn_stats(
                                bnq[:, h * 6:h * 6 + 6],
                                Fq[:, t * DIM + h * 128:
                                   t * DIM + h * 128 + 128])
                            nc.vector.bn_aggr(qa[:, 2 * c:2 * c + 2],
                                              bnq[:, h * 6:h * 6 + 6])
                            nc.vector.tensor_scalar_mul(
                                qmean[:, c:c + 1],
                                qa[:, 2 * c:2 * c + 1], -1.0)
                            fsl = slice(t * DIM + h * 128,
                                        t * DIM + h * 128 + 128)
                            qc = qcpool.tile([128, 128], BF16, tag="qc",
                                             name="qc")
                            nc.vector.tensor_scalar(qc[:], Fq[:, fsl],
                                                    qmean[:, c:c + 1], None,
                                                    ALU.add)
                            qc_tiles[(t, h)] = qc
                    else:
                        for h in range(H):
                            nc.vector.bn_stats(
                                bnq[:, h * 6:h * 6 + 6],
                                Fq[:, t * DIM + h * 128:
                                   t * DIM + h * 128 + 128])
                        for h in range(H):
                            c = t * 8 + h
                            nc.vector.bn_aggr(qa[:, 2 * c:2 * c + 2],
                                              bnq[:, h * 6:h * 6 + 6])
                        nc.vector.tensor_scalar_mul(
                            qmean[:, t * 8:t * 8 + 8],
                            qa[:, 16 * t:16 * t + 16:2], -1.0)
                        for h in range(H):
                            c = t * 8 + h
                            fsl = slice(t * DIM + h * 128,
                                        t * DIM + h * 128 + 128)
                            qc = qcpool.tile([128, 128], BF16, tag="qc",
                                             name="qc")
                            nc.gpsimd.tensor_scalar(qc[:], Fq[:, fsl],
                                                    qmean[:, c:c + 1], None,
                                                    ALU.add)
                            qc_tiles[(t, h)] = qc
                elif i == 1:
                    for h in range(H):
                        nc.vector.bn_stats(
                            bnk[:, h * 6:h * 6 + 6],
                            Fk[:, t * DIM + h * 128:t * DIM + h * 128 + 128])
                    for h in range(H):
                        c = t * 8 + h
                        nc.vector.bn_aggr(ka[:, 2 * c:2 * c + 2],
                                          bnk[:, h * 6:h * 6 + 6])
            nc.scalar.dma_start(WoT[:, t * DIM:(t + 1) * DIM],
                                WoT_d[:, t * DIM:(t + 1) * DIM])

            if t in (3, 7):
                jh = t // 4
                sl = slice(32 * jh, 32 * jh + 32)
                derived(qa, qninv, qrb, jh)
                derived(ka, kninv, kr, jh, n_out=kn)
                nc.vector.tensor_tensor(krkn[:, sl], kr[:, sl], kn[:, sl],
                                        op=ALU.mult)
                tts = range(4) if jh == 0 else range(7, 8)
                for tt in tts:
                    for h in range(H):
                        c = tt * 8 + h
                        fsl = slice(tt * DIM + h * 128,
                                    tt * DIM + h * 128 + 128)
                        nc.vector.tensor_scalar(
                            Fv[:, fsl], Fv[:, fsl],
                            kninv[:, c:c + 1], None, ALU.mult)
            # task-0 attention-core work rides inside phase 1: its PE bursts
            # interleave with the dense projection stream and its evictions
            # land on engine slack
            if t == 5:
                emit_gk(0)
                emit_fqscale(0)
                emit_mmv(0)
            elif t == 6:
                emit_trp(0)

    # ---- remaining constants (first used after phase 1) ----
    ident8 = consts.tile([8, 8], F32)
    nc.sync.dma_start(ident8[:], ident_d[0:8, 0:8])
    ones = consts.tile([128, 8], F32)
    nc.sync.dma_start(ones[:], ones_d[:, 0:8])
    ones8 = consts.tile([1, 8], F32)
    nc.sync.dma_start(ones8[:], ones_d[0:1, 0:8])
    mask_nd = consts.tile([128, H * 128], BF16)
    nc.scalar.dma_start(mask_nd[:], mask_d[:])
    wp1T = consts.tile([128, 256], F32)
    nc.scalar.dma_start(wp1T[:], wp1T_d[:])
    wp2T = consts.tile([128, 3], F32)
    nc.scalar.dma_start(wp2T[:], wp2T_d[:])
    b1row = consts.tile([1, 128], F32)
    nc.scalar.dma_start(b1row[:], b1_d[:])
    gbc = consts.tile([8, 128], F32)
    nc.scalar.dma_start(gbc[:], gbc_d[:])
    bbc = consts.tile([8, 128], F32)
    nc.scalar.dma_start(bbc[:], bbc_d[:])
    b2bc = consts.tile([8, 3], F32)
    nc.scalar.dma_start(b2bc[:], b2bc_d[:])

    # ======== post-phase-1: task-1 region + collectives ========
    ps_small = pre.enter_context(tc.tile_pool(name="ps_small", bufs=1,
                                              space="PSUM"))
    emit_gk(1)
    gk_sb = trsc.tile([128, 16], BF16, tag="gksb", name="gk_sb")
    nc.scalar.copy(gk_sb[:], gk_ps[:])
    nc.sync.dma_start(ar_in_g[:], gk_sb[:])
    if n_cores > 1:
        nc.gpsimd.collective_compute(
            "AllReduce", ALU.add,
            replica_groups=[list(range(n_cores))],
            ins=[ar_in_g.opt()], outs=[ar_out_g.opt()])
    else:
        nc.sync.dma_start(ar_out_g[:], ar_in_g[:])
    arg = late.tile([128, 16], BF16)
    nc.scalar.dma_start(arg[:], ar_out_g[:])

    # feat_corr partials per head, shipped as they complete
    for h in range(H):
        fc_ps = ps_fc.tile([128, 128], F32, tag="fc128", name="fc_ps")
        for t in range(NT):
            nc.tensor.matmul(fc_ps[:], qc_tiles[(t, h)][:],
                             qc_tiles[(t, h)][:],
                             start=(t == 0), stop=(t == NT - 1))
        fc_sb = trsc.tile([128, 128], BF16, tag="fcsb", name="fc_sb")
        nc.scalar.copy(fc_sb[:], fc_ps[:])
        nc.sync.dma_start(ar_in_fc[:, h * 128:(h + 1) * 128], fc_sb[:])
    if n_cores > 1:
        nc.gpsimd.collective_compute(
            "AllReduce", ALU.add,
            replica_groups=[list(range(n_cores))],
            ins=[ar_in_fc.opt()], outs=[ar_out_fc.opt()])
    else:
        nc.sync.dma_start(ar_out_fc[:], ar_in_fc[:])
    ar = late.tile([128, H * 128], BF16)
    nc.sync.dma_start(ar[:], ar_out_fc[:])

    emit_fqscale(1)
    emit_mmv(1)

    # ======== Phase 3a: weight predictor (needs only gk slice) ========
    featsq = stat1.tile([128, 8], F32)
    nc.vector.tensor_scalar_mul(featsq[:], arg[:, 0:8], 1.0 / TOK_ALL)
    featsk = stat1.tile([128, 8], F32)
    nc.vector.tensor_scalar_mul(featsk[:], arg[:, 8:16], 1.0 / TOK_ALL)
    h1_ps = ps_small.tile([8, 128], F32, tag="sm", name="h1_ps")
    nc.tensor.matmul(h1_ps[:], featsq[:], wp1T[:, 0:128], start=True,
                     stop=False)
    nc.tensor.matmul(h1_ps[:], featsk[:], wp1T[:, 128:256], start=False,
                     stop=False)
    nc.tensor.matmul(h1_ps[:], ones8[:], b1row[:], start=False, stop=True)
    h1 = stat1.tile([8, 128], F32)
    nc.scalar.copy(h1[:], h1_ps[:])
    w_mu = stat1.tile([8, 4], F32)
    sq8 = stat1.tile([8, 128], F32)
    nc.vector.reduce_sum(w_mu[:, 0:1], h1[:], axis=AX.X)
    nc.vector.tensor_scalar_mul(w_mu[:, 0:1], w_mu[:, 0:1], 1.0 / D)
    nc.scalar.activation(sq8[:], h1[:], AF.Square, accum_out=w_mu[:, 1:2])
    nc.vector.tensor_scalar_mul(w_mu[:, 1:2], w_mu[:, 1:2], 1.0 / D)
    nc.vector.tensor_tensor(w_mu[:, 2:3], w_mu[:, 0:1], w_mu[:, 0:1],
                            op=ALU.mult)
    nc.vector.tensor_tensor(w_mu[:, 2:3], w_mu[:, 1:2], w_mu[:, 2:3],
                            op=ALU.subtract)
    nc.scalar.activation(w_mu[:, 3:4], w_mu[:, 2:3], AF.Sqrt, bias=eps[0:8, :])
    nc.vector.reciprocal(w_mu[:, 3:4], w_mu[:, 3:4])
    h1n = stat1.tile([8, 128], F32)
    nc.vector.tensor_scalar(h1n[:], h1[:], w_mu[:, 0:1], w_mu[:, 3:4],
                            ALU.subtract, ALU.mult)
    nc.vector.tensor_tensor(h1n[:], h1n[:], gbc[:], op=ALU.mult)
    nc.vector.tensor_tensor(h1n[:], h1n[:], bbc[:], op=ALU.add)
    nc.vector.tensor_scalar_max(h1n[:], h1n[:], 0.0)
    h1T_ps = ps_small.tile([128, 8], F32, tag="sm", name="h1T_ps")
    nc.tensor.transpose(h1T_ps[:], h1n[:], ident8[:])
    h1T = stat1.tile([128, 8], F32)
    nc.scalar.copy(h1T[:], h1T_ps[:])
    lg_ps = ps_small.tile([8, 3], F32, tag="sm", name="lg_ps")
    nc.tensor.matmul(lg_ps[:], h1T[:], wp2T[:], start=True, stop=True)
    lg = stat1.tile([8, 8], F32)
    nc.scalar.copy(lg[:, 0:3], lg_ps[:])
    nc.vector.tensor_tensor(lg[:, 0:3], lg[:, 0:3], b2bc[:], op=ALU.add)
    # logits are O(1): skip the (mathematically redundant) max-subtraction
    nc.scalar.activation(lg[:, 0:3], lg[:, 0:3], AF.Exp)
    nc.vector.reduce_sum(lg[:, 4:5], lg[:, 0:3], axis=AX.X)
    nc.vector.reciprocal(lg[:, 4:5], lg[:, 4:5])
    nc.vector.tensor_scalar(lg[:, 0:3], lg[:, 0:3], lg[:, 4:5], None,
                            ALU.mult)



    # ======== Phase 3b: decorr scale (needs feat_corr block) ========
    ssq = stat1.tile([128, 8], F32)
    msk = late.tile([128, H * 128], BF16)
    nc.vector.tensor_tensor(msk[:], ar[:], mask_nd[:], op=ALU.mult)
    sqf = late.tile([128, H * 128], F32)
    nc.scalar.activation(sqf[:], msk[:], AF.Square)
    nc.vector.reduce_sum(ssq[:],
                         sqf[:].rearrange("p (h d) -> p h d", h=8),
                         axis=AX.X)
    ss_ps = ps_small.tile([8, 8], F32, tag="sm", name="ss_ps")
    nc.tensor.matmul(ss_ps[:], ssq[:], ones[:, 0:8], start=True, stop=True)
    dsc = stat1.tile([8, 8], F32)
    nc.scalar.activation(dsc[:, 0:1], ss_ps[0:8, 0:1], AF.Sqrt)
    nc.scalar.activation(dsc[:, 1:2], dsc[:, 0:1], AF.Exp,
                         scale=-5.0 / (D * D * TOK_ALL))

    # alpha = w0 + w1*dsc ; wsc = w2/alpha ; flat row [alpha(8) | wsc(8)]
    aw = stat1.tile([8, 4], F32)
    nc.vector.tensor_tensor(aw[:, 0:1], lg[:, 1:2], dsc[:, 1:2], op=ALU.mult)
    nc.vector.tensor_tensor(aw[:, 0:1], aw[:, 0:1], lg[:, 0:1], op=ALU.add)
    nc.vector.reciprocal(aw[:, 2:3], aw[:, 0:1])
    nc.vector.tensor_tensor(aw[:, 1:2], lg[:, 2:3], aw[:, 2:3], op=ALU.mult)
    awT_ps = ps_small.tile([1, 8], F32, tag="sm", name="awT_ps")
    nc.tensor.transpose(awT_ps[:], aw[:, 0:1], ident8[:])
    awTa = stat1.tile([1, 8], F32)
    nc.scalar.copy(awTa[:], awT_ps[:])
    awT_ps2 = ps_small.tile([1, 8], F32, tag="sm", name="awT_ps2")
    nc.tensor.transpose(awT_ps2[:], aw[:, 1:2], ident8[:])
    awTb = stat1.tile([1, 8], F32)
    nc.scalar.copy(awTb[:], awT_ps2[:])
    abc = stat1.tile([128, 8], F32)
    nc.gpsimd.partition_broadcast(abc[:], awTa[:])
    wscbc = stat1.tile([128, 8], F32)
    nc.gpsimd.partition_broadcast(wscbc[:], awTb[:])
    pre.close()

    # ======== Phase 4b + 5: scaled attention + output projection ========
    with tc.tile_pool(name="ph4", bufs=2) as ph4, \
         tc.tile_pool(name="o1pool", bufs=10) as o1pool, \
         tc.tile_pool(name="ps_o1", bufs=3, space="PSUM") as ps_o1, \
         tc.tile_pool(name="ps_p5", bufs=2, space="PSUM") as ps_p5:
        o1_tiles = {}
        for j in range(NTASK):
            for h in range(H):
                # mv row scaled by ww/alpha (tiny); alpha applied at eviction
                mvw = ph4.tile([1, 128], BF16, tag="mvw", name="mvw")
                nc.vector.tensor_scalar(mvw[:], mv_raw[(h, j)][:],
                                        wscbc[0:1, h:h + 1], None,
                                        ALU.mult)
                o1_ps = ps_o1.tile([128, 512], F32, tag="o1", name="o1_ps")
                nc.tensor.matmul(o1_ps[:], mm_raw[(h, j)][:],
                                 fqT_tiles[(h, j)][:], start=True, stop=False)
                nc.tensor.matmul(o1_ps[:], mvw[:], wqr_tiles[(h, j)][:],
                                 start=False, stop=True)
                o1 = o1pool.tile([128, 512], BF16, tag="o1sb", name="o1_sb")
                if h % 2 == 0:
                    nc.vector.tensor_scalar(o1[:], o1_ps[:], abc[:, h:h + 1],
                                            None, ALU.mult)
                else:
                    nc.scalar.mul(o1[:], o1_ps[:], abc[:, h:h + 1])
                o1_tiles[(h, j)] = o1

            # ---- output projection for this task ----
            for t in range(4 * j, 4 * j + 4):
                ti = t % 4
                for half in range(2):
                    o = half * 512
                    op_ps = ps_p5.tile([128, 512], F32, tag="p5",
                                       name="op_ps")
                    for h in range(H):
                        nc.tensor.matmul(
                            op_ps[:],
                            o1_tiles[(h, j)][:, ti * 128:(ti + 1) * 128],
                            WoT[:, h * DIM + o: h * DIM + o + 512],
                            start=(h == 0),
                            stop=(h == H - 1 and not has_bias))
                    if has_bias:
                        nc.tensor.matmul(op_ps[:], onebf_row[:],
                                         bout[:, o:o + 512],
                                         start=False, stop=True)
                    ysb = ph4.tile([128, 512], F32, tag="ysb", name="ysb")
                    if j == 1 and (t + half) % 2 == 1:
                        nc.vector.tensor_copy(ysb[:], op_ps[:])
                    else:
                        nc.scalar.copy(ysb[:], op_ps[:])
                    qy = nc.sync if (t + half) % 2 == 0 else nc.scalar
                    qy.dma_start(y[t * 128:(t + 1) * 128, o:o + 512],
                                 ysb[:])


_BUILT = {}


def _build(n_cores=N_CORES, has_bias=False):
    key = (n_cores, has_bias)
    if key in _BUILT:
        return _BUILT[key]
    nc = bacc.Bacc("TRN2", target_bir_lowering=False, debug=False,
                   num_devices=n_cores)
    in_specs = [
        ("xn_q", [T, DIM], BF16), ("xn_k", [T, DIM], BF16),
        ("xn_v", [T, DIM], BF16),
        ("xT_q", [128, NT * DIM], BF16), ("xT_k", [128, NT * DIM], BF16),
        ("xT_v", [128, NT * DIM], BF16),
        ("Wp", [128, 8 * DIM], BF16), ("WoT", [128, 8 * DIM], BF16),
        ("vrow", [1, DIM], BF16), ("bout", [1, DIM], BF16),
        ("ones", [128, 128], F32), ("onesbf", [128, 8], BF16),
        ("identbf", [128, 128], BF16), ("ident", [128, 128], F32),
        ("mask", [128, 1024], BF16),
        ("wp1T", [128, 256], F32), ("wp2T", [128, 3], F32),
        ("b1row", [1, 128], F32),
        ("gbc", [8, 128], F32), ("bbc", [8, 128], F32), ("b2bc", [8, 3], F32),
    ]
    in_aps = [nc.dram_tensor(n, s, dt, kind="ExternalInput").ap()
              for n, s, dt in in_specs]
    y_ap = nc.dram_tensor("y", [T, DIM], F32, kind="ExternalOutput").ap()
    with tile.TileContext(nc) as tc:
        attn_kernel(tc, [y_ap], in_aps, n_cores=n_cores, has_bias=has_bias)
    nc.compile()
    _BUILT[key] = nc
    return nc


def _bf(x):
    import ml_dtypes
    return np.asarray(x, dtype=ml_dtypes.bfloat16)


def kernel(q, k, v, ln_g, ln_b, w_in, wp_w1, wp_b1, wp_ln_g, wp_ln_b,
           wp_w2, wp_b2, w_out, b_out):
    q = np.asarray(q, dtype=np.float32)
    k = np.asarray(k, dtype=np.float32)
    v = np.asarray(v, dtype=np.float32)
    ln_g = np.asarray(ln_g, np.float32); ln_b = np.asarray(ln_b, np.float32)
    w_in = np.asarray(w_in, np.float32); w_out = np.asarray(w_out, np.float32)
    b_out = np.asarray(b_out, np.float32)
    wp_w1 = np.asarray(wp_w1, np.float32); wp_b1 = np.asarray(wp_b1, np.float32)
    wp_ln_g = np.asarray(wp_ln_g, np.float32)
    wp_ln_b = np.asarray(wp_ln_b, np.float32)
    wp_w2 = np.asarray(wp_w2, np.float32); wp_b2 = np.asarray(wp_b2, np.float32)

    # host weight prep: fold LN gain into W, then column-center so x @ Wp
    # carries the -mu*sum(g*W) correction implicitly
    W = w_in.T                                     # [DIM, HD]
    Wp = (ln_g[:, None] * W)
    Wp = Wp - Wp.mean(axis=0, keepdims=True)
    vrow = (ln_b @ W)[None, :]
    has_bias = bool(np.any(ln_b != 0.0) or np.any(b_out != 0.0))
    Wp_t = np.ascontiguousarray(
        Wp.reshape(8, 128, 2, 512).transpose(1, 0, 2, 3)).reshape(128, -1)
    WoT = np.ascontiguousarray(
        w_out.T.reshape(8, 128, DIM).transpose(1, 0, 2)).reshape(128, -1)
    shared = {
        "Wp": _bf(Wp_t), "WoT": _bf(WoT), "vrow": _bf(vrow),
        "bout": _bf(b_out[None, :]),
        "ones": np.ones((128, 128), np.float32),
        "onesbf": _bf(np.ones((128, 8), np.float32)),
        "identbf": _bf(np.eye(128, dtype=np.float32)),
        "ident": np.eye(128, dtype=np.float32),
        "mask": _bf(np.tile((1.0 - np.eye(128)).astype(np.float32), (1, 8))),
        "wp1T": np.ascontiguousarray(wp_w1.T.reshape(2, 128, 128)
                                     .transpose(1, 0, 2)).reshape(128, 256),
        "wp2T": np.ascontiguousarray(wp_w2.T),
        "b1row": wp_b1[None, :],
        "gbc": np.tile(wp_ln_g[None, :], (8, 1)),
        "bbc": np.tile(wp_ln_b[None, :], (8, 1)),
        "b2bc": np.tile(wp_b2[None, :], (8, 1)),
    }
    for kk in ("ones", "ident", "wp1T", "wp2T", "b1row", "gbc", "bbc",
               "b2bc"):
        shared[kk] = np.ascontiguousarray(shared[kk], np.float32)

    qf = q.reshape(QB * N, DIM)
    kf = k.reshape(QB * N, DIM)
    vf = v.reshape(QB * N, DIM)
    in_maps = []
    for c in range(N_CORES):
        sl = slice(c * T, (c + 1) * T)
        m = dict(shared)
        for nm, arr in (("q", qf[sl]), ("k", kf[sl]), ("v", vf[sl])):
            m[f"xn_{nm}"] = _bf(np.ascontiguousarray(arr))
            m[f"xT_{nm}"] = _bf(np.ascontiguousarray(
                arr.reshape(NT, 128, 8, 128).transpose(3, 0, 2, 1)
            ).reshape(128, NT * DIM))
        in_maps.append(m)

    nc = _build(has_bias=has_bias)
    res = bass_utils.run_bass_kernel_spmd(nc, in_maps,
                                          core_ids=list(range(N_CORES)))
    global LAST_RESULTS
    LAST_RESULTS = res
    out = np.concatenate([np.asarray(r["y"], np.float32)
                          for r in res.results], axis=0)
    return out.reshape(QB, N, DIM)


LAST_RESULTS = None
